# revision 1
# baseline (speedup 1.0000x reference)
"""Trainium2 Bass kernel for nn_Decoder_22703197127089 (moe_routing).

Only the last token survives to the output (h[:, -1, :] is taken after the
MoE block), so per sample we need: conv patch-embed for all 1023 tokens
(feeds K/V), folded-LN attention row for the last token, MoE + LN2 + head
for one token.

v2 layout/scheduling notes vs the v1 baseline:
  - X is deinterleaved HOST-SIDE into even/odd columns: Xd (128, 6144) with
    rows 0-63 = X[c, 2j], rows 64-127 = X[c, 2j+1].  A K=128 conv chunk k
    (positions 2k, 2k+1) then reads cols 6n+k -- X is DMA'd ONCE (no
    SBUF->SBUF shifted duplicate), halving DMA volume.
  - All constants ride in two packed tensors (one bf16, one f32) via HWDGE.
  - The whole middle section computes in bf16 (1 cyc/row matmuls).
  - rstd = exp(-0.5*ln(var+eps)); with Square/Exp/Ln/Copy all in the
    natural_log_exp_and_others act set, only one act-table load is needed.
  - Scores use host-precomputed A = qw.T@kw: two raw score streams
    s1 = (A h_last)^T h0, s2 = arow^T h0 need no LN stats, so the per-token
    correction is pure DVE work after the stats pipeline.
  - The v-side LN correction g rides as a second column block in the et/gt
    Z-reduction matmuls; MoE router softmax normalization is dropped
    entirely (LN2 is scale-invariant).
  - PE warmed up with junk matmuls during the initial DMA so conv runs at
    full p-state; emission hand-interleaves pair-0 attention with pair-1
    conv and pair-0 tail with pair-1 attention.

Sharding: data-parallel over batch B=32 across 8 cores (4 samples/core),
host gathers (4, 96) per-core outputs.
"""

import numpy as np

import concourse.bass as bass
import concourse.mybir as mybir
import concourse.tile as tile
from concourse import bacc
from concourse.bass_utils import run_bass_kernel_spmd

F32 = mybir.dt.float32
BF16 = mybir.dt.bfloat16
AF = mybir.ActivationFunctionType
OP = mybir.AluOpType

B, C, L = 32, 64, 12288
D = 64
E = 8
TOPK = 4
P, S = 24, 12
PRED = 96
N = (L - P) // S + 1  # 1023
NCORES = 8
SPC = B // NCORES     # 4 samples per core
NPAIR = SPC // 2      # 2
NCH = (C * P) // 128  # 12 contraction chunks of K=128
NT = 1024             # padded token dim (col 1023 zeroed)
NJ = 8                # 128-token chunks
EPS = 1e-5
LH = 6144             # deinterleaved X columns
XSPLIT = 3072         # X half-DMA split: m1 reads only [3072:6144]; m0
                      # reads [0:3078] (its last patch peeks 6 cols into h1,
                      # which always lands first)
MCS = (512, 511)      # conv m-chunk sizes (patches)

# ---- CB (bf16) column map ----
CB_WSB = 0            # 12 * 64 conv weight chunks
CB_AT = 768           # A^T = qw.T @ kw, doubled rows        (64)
CB_VWT = 832          # vw.T doubled                          (64)
CB_SEL = 896          # selab                                 (2)
CB_AROW = 898         # arow doubled                          (1)
CB_ONE = 899          # ones column                           (1)
CB_LASTM = 900        # ones, 0 at partition 127              (1)
CB_WEXP = 904         # experts (rows 0:65)                   (512)
CBW = 1416

# ---- CF (f32) column map ----
CF_PEBT = 0           # (pe + conv_b).T doubled               (1024, col 1023 zero)
CF_OWT = 1024         # ow.T rows 0:64                        (64)
CF_MOWT = 1088        # moe_out_w.T rows 0:64                 (64)
CF_OUTWT = 1152       # out_w.T rows 0:64                     (96)
CF_RWT = 1248         # router_w.T rows 0:64                  (8)
CF_SV = 1256          # vw.sum(1) rows 0:64                   (1)
CF_OH = 1257          # onehot at partition 126               (1)
CF_ID4 = 1258         # eye(4) rows 0:4                       (4)
CF_G32 = 1262         # Z/g parity grouping rows 0:48         (4)
CF_ONESR = 1266       # ones row 0                            (128)
CF_ONE64 = 1394       # ones rows 0:64                        (1)
CF_OCS = 1395         # out_w.T colsums, doubled rows 0:2     (96)
CF_LASTM = 1491       # ones, 0 at partition 127              (1)
CFW = 1492


def _pos_encoding_np(n, d):
    pos = np.arange(n, dtype=np.float32)[:, None]
    div = np.exp(np.arange(0, d, 2, dtype=np.float32)
                 * (np.float32(-np.log(np.float32(10000.0))) / np.float32(d)))
    pe = np.zeros((n, d), np.float32)
    pe[:, 0::2] = np.sin(pos * div)
    pe[:, 1::2] = np.cos(pos * div)
    return pe


def build_nc():
    nc = bacc.Bacc("TRN2", target_bir_lowering=False, debug=False,
                   num_devices=NCORES)

    Xd = nc.dram_tensor("Xd", [SPC, 128, LH], BF16, kind="ExternalInput")
    CBt = nc.dram_tensor("CB", [128, CBW], BF16, kind="ExternalInput")
    CFt = nc.dram_tensor("CF", [128, CFW], F32, kind="ExternalInput")
    Yout = nc.dram_tensor("Yout", [SPC, PRED], F32, kind="ExternalOutput")

    with tile.TileContext(nc) as tc:
        with (
            tc.tile_pool(name="const", bufs=1) as pc,
            tc.tile_pool(name="xp", bufs=4) as xp,
            tc.tile_pool(name="hp", bufs=2) as hp,
            tc.tile_pool(name="sm", bufs=2) as sm,
            tc.tile_pool(name="vp", bufs=2) as vp,
            tc.tile_pool(name="ps", bufs=1, space="PSUM") as ps,
        ):
            # ---------------- constants / warmup ----------------
            cb = pc.tile([128, CBW], BF16, tag="cb")
            cf = pc.tile([128, CFW], F32, tag="cf")
            junk = pc.tile([128, 512], BF16, tag="junk")
            epsb = pc.tile([128, 1], F32, tag="epsb")
            nc.vector.memset(junk[:], 0.25)
            nc.vector.memset(epsb[:], EPS)

            # Pre-load the one act-function set covering Square/Exp/Ln/Copy
            # so the act-table pass never inserts another (greedy first-match
            # would otherwise thrash between exp_and_others and natural_log).
            from concourse.hw_specs import get_activation_tables
            _set_id = list(get_activation_tables(nc.m.arch)).index(
                "natural_log_exp_and_others")
            nc.scalar.add_instruction(mybir.InstLoadActFuncSet(
                name=nc.get_next_instruction_name(), ins=[], outs=[],
                act_func_set_id=_set_id))

            # DMA order: small consts first, then per-sample h1 halves before
            # h0 halves (m1 holds the last token + j7 stats, so each pair's
            # serial scalar path starts as early as possible); the expert
            # weights (only needed by the tail) ride last.
            xdt = []
            for s in range(SPC):
                t = xp.tile([128, LH], BF16, tag="xd", name="xd%d" % s)
                xdt.append(t)

            def xh(s, half):
                if half == 0:
                    nc.sync.dma_start(xdt[s][:, 0:XSPLIT],
                                      Xd.ap()[s][:, 0:XSPLIT])
                else:
                    nc.sync.dma_start(xdt[s][:, XSPLIT:LH],
                                      Xd.ap()[s][:, XSPLIT:LH])

            nc.sync.dma_start(cb[:, 0:CB_WEXP], CBt.ap()[:, 0:CB_WEXP])
            xh(0, 1)
            nc.sync.dma_start(cf[:], CFt.ap())
            xh(0, 0)
            xh(1, 1)
            xh(1, 0)
            xh(2, 1)
            xh(3, 1)
            xh(2, 0)
            xh(3, 0)
            nc.sync.dma_start(cb[:, CB_WEXP:CBW], CBt.ap()[:, CB_WEXP:CBW])

            wsb = cb[:, CB_WSB:CB_WSB + 768].rearrange("p (k d) -> p k d", k=NCH)
            at2 = cb[:, CB_AT:CB_AT + 64]
            vwt2 = cb[:, CB_VWT:CB_VWT + 64]
            selab = cb[:, CB_SEL:CB_SEL + 2]
            arow2 = cb[:, CB_AROW:CB_AROW + 1]
            onesb = cb[:, CB_ONE:CB_ONE + 1]
            lastmf = cf[:, CF_LASTM:CF_LASTM + 1]
            wexpb = cb[0:D + 1, CB_WEXP:CB_WEXP + E * D]
            pebt = cf[:, CF_PEBT:CF_PEBT + NT]
            owt = cf[0:D, CF_OWT:CF_OWT + D]
            mowt = cf[0:D, CF_MOWT:CF_MOWT + D]
            outwt = cf[0:D, CF_OUTWT:CF_OUTWT + PRED]
            rwt = cf[0:D, CF_RWT:CF_RWT + E]
            svcol = cf[0:D, CF_SV:CF_SV + 1]
            onehotf = cf[:, CF_OH:CF_OH + 1]
            id4 = cf[0:4, CF_ID4:CF_ID4 + 4]
            g48 = cf[0:48, CF_G32:CF_G32 + 4]
            onesr = cf[0:1, CF_ONESR:CF_ONESR + 128]
            ones64 = cf[0:D, CF_ONE64:CF_ONE64 + 1]
            ocs4 = cf[0:SPC, CF_OCS:CF_OCS + PRED]

            # PE warmup: keep the tensor engine busy during initial DMA so
            # real matmuls dispatch at full p-state.
            jp = ps.tile([128, 512], F32, tag="v0p", name="junkp", bufs=2)
            jp2 = ps.tile([128, 512], F32, tag="v0p", name="junkp2", bufs=2)
            for i in range(7):
                t = jp if i % 2 == 0 else jp2
                nc.tensor.matmul(t[:, 0:512], lhsT=junk[:, 0:128],
                                 rhs=junk[:, 0:512], start=True, stop=True)

            # Shared attention-output tiles (row 64 = 1.0 feeds expert bias)
            haf = pc.tile([D + 1, SPC], F32, tag="haf")
            hab = pc.tile([D + 1, SPC], BF16, tag="hab")
            nc.vector.memset(haf[D:D + 1, :], 1.0)
            nc.vector.memset(hab[D:D + 1, :], 1.0)

            # Per-pair state dicts + shared tail state
            PR = [dict() for _ in range(NPAIR)]
            TL = {}

            # ---------------- emission helpers ----------------
            def conv_m(pair, s01, mi):
                """12 accumulation matmuls for sample (2*pair+s01), m-chunk mi."""
                st = PR[pair]
                s = 2 * pair + s01
                n0 = 0 if mi == 0 else MCS[0]
                nn = MCS[mi]
                key = "cps%d" % mi
                if key not in st:
                    st[key] = ps.tile([128, 512], F32, tag="cps",
                                      name="cps_p%d_m%d" % (pair, mi), bufs=2)
                cps = st[key]
                o = 64 * s01
                xv = xdt[s][:].rearrange("p (n s) -> p n s", s=6)
                for k in range(NCH):
                    q, r = divmod(k, 6)
                    nc.tensor.matmul(
                        cps[o:o + 64, 0:nn],
                        lhsT=wsb[:, k, :],
                        rhs=xv[:, n0 + q:n0 + q + nn, r],
                        start=(k == 0), stop=(k == NCH - 1))

            def evict_m(pair, mi):
                """psum + pebt -> h0b (bf16) on DVE; Square -> sq on Act."""
                st = PR[pair]
                if "h0b" not in st:
                    st["h0b"] = hp.tile([128, NT], BF16, tag="h0b",
                                        name="h0b_p%d" % pair)
                    st["sq"] = hp.tile([128, NT], BF16, tag="sq",
                                       name="sq_p%d" % pair)
                    nc.vector.memset(st["h0b"][:, N:NT], 0.0)
                    nc.vector.memset(st["sq"][:, N:NT], 0.0)
                n0 = 0 if mi == 0 else MCS[0]
                nn = MCS[mi]
                cps = st["cps%d" % mi]
                nc.vector.tensor_add(st["h0b"][:, n0:n0 + nn], cps[:, 0:nn],
                                     pebt[:, n0:n0 + nn])
                nc.scalar.activation(st["sq"][:, n0:n0 + nn],
                                     st["h0b"][:, n0:n0 + nn], AF.Square)

            def alloc_attn(pair):
                st = PR[pair]
                st["stp"] = ps.tile([128, 64], F32, tag="stp",
                                    name="stp_p%d" % pair, bufs=2)
                st["misc"] = ps.tile([128, 160], F32, tag="misc",
                                     name="misc_p%d" % pair, bufs=2)

            def stats_mm(pair, js):
                """Per-token sums (cols 2j+t) and sum-of-squares (32+...)."""
                st = PR[pair]
                h0b, sq, stp = st["h0b"], st["sq"], st["stp"]
                for j in js:
                    nc.tensor.matmul(stp[:, 2 * j:2 * j + 2],
                                     lhsT=h0b[:, 128 * j:128 * j + 128],
                                     rhs=selab, start=True, stop=True)
                    nc.tensor.matmul(stp[:, 32 + 2 * j:32 + 2 * j + 2],
                                     lhsT=sq[:, 128 * j:128 * j + 128],
                                     rhs=selab, start=True, stop=True)

            def q0_mm(pair):
                st = PR[pair]
                misc = st["misc"]
                for s01 in range(2):
                    o = 64 * s01
                    nc.tensor.matmul(misc[o:o + 64, 0:1],
                                     lhsT=at2[o:o + 64, :],
                                     rhs=st["h0b"][o:o + 64, N - 1:N],
                                     start=True, stop=True)

            def q0_evict(pair):
                # the 1/sqrt(D)=1/8 score scale is folded here (and into the
                # host-side arow), so the assembly needs no extra multiply
                st = PR[pair]
                st["q0sb"] = sm.tile([128, 1], BF16, tag="q0sb", name="q0sb_t")
                nc.vector.tensor_scalar_mul(st["q0sb"][:], st["misc"][:, 0:1],
                                            0.125)

            def cA_mm(pair):
                # cA = sum(q0) per sample -> misc[0:1, 14:16]
                st = PR[pair]
                nc.tensor.matmul(st["misc"][0:1, 14:16], lhsT=st["q0sb"][:],
                                 rhs=selab, start=True, stop=True)

            def v0_mm(pair, s01, js):
                st = PR[pair]
                key = "v0p%d" % s01
                if key not in st:
                    st[key] = ps.tile([128, 512], F32, tag="v0p",
                                      name="v0_p%d_s%d" % (pair, s01), bufs=2)
                v0p = st[key]
                o = 64 * s01
                for j in js:
                    nc.tensor.matmul(v0p[:, 64 * j:64 * j + 64],
                                     lhsT=st["h0b"][o:o + 64, 128 * j:128 * j + 128],
                                     rhs=vwt2[o:o + 64, :],
                                     start=True, stop=True)

            def v0_evict(pair, s01):
                st = PR[pair]
                st["v0sb%d" % s01] = vp.tile([128, 512], BF16, tag="v0sb", name="v0sb_t")
                if s01 == 0:
                    nc.scalar.copy(st["v0sb%d" % s01][:], st["v0p%d" % s01][:])
                else:
                    nc.vector.tensor_copy(st["v0sb%d" % s01][:],
                                          st["v0p%d" % s01][:])

            def j7_path_a(pair):
                """Raw last-token stats -> ext -> transpose (PE part 1)."""
                st = PR[pair]
                stp, misc = st["stp"], st["misc"]
                st["j7sb"] = sm.tile([128, 4], F32, tag="j7sb", name="j7sb_t")
                # cols {14,15,46,47}: sums and sumsq for tokens 1022(A/B)
                nc.vector.tensor_copy(
                    st["j7sb"][:].rearrange("p (a b) -> p a b", a=2),
                    stp[:].rearrange("p (a b) -> p a b", a=2)[:, :, 14:16])
                nc.tensor.matmul(misc[0:4, 2:3], lhsT=st["j7sb"][:],
                                 rhs=onehotf, start=True, stop=True)

            def j7_path_b(pair):
                st = PR[pair]
                misc = st["misc"]
                st["e4"] = sm.tile([4, 1], F32, tag="e4", name="e4_t")
                nc.vector.tensor_copy(st["e4"][:], misc[0:4, 2:3])
                nc.tensor.matmul(misc[0:1, 4:8], lhsT=st["e4"][:], rhs=id4,
                                 start=True, stop=True)

            def j7_path_c(pair):
                """Scalar DVE/Act pipeline to mb = [muL_A muL_B rL_A rL_B]."""
                st = PR[pair]
                misc = st["misc"]
                mb = sm.tile([1, 4], F32, tag="mb", name="mb_t")
                st["mb"] = mb
                tv = sm.tile([1, 4], F32, tag="tv", name="tv_t")
                # tv = raw/64 : [sumA sumB sqA sqB] -> [muA muB exA exB]
                nc.vector.tensor_scalar_mul(tv[:], misc[0:1, 4:8], 1.0 / D)
                nc.vector.tensor_copy(mb[0:1, 0:2], tv[0:1, 0:2])
                # var = ex - mu^2
                t2 = sm.tile([1, 2], F32, tag="t2", name="t2_t")
                nc.vector.tensor_mul(t2[:], tv[0:1, 0:2], tv[0:1, 0:2])
                nc.vector.tensor_sub(t2[:], tv[0:1, 2:4], t2[:])
                # rl = exp(-0.5 * ln(var + eps))
                nc.scalar.activation(t2[:], t2[:], AF.Ln, bias=epsb[0:1, :])
                nc.scalar.activation(mb[0:1, 2:4], t2[:], AF.Exp, scale=-0.5)

            def bcast1_mm(pair):
                st = PR[pair]
                nc.tensor.matmul(st["misc"][:, 8:12], lhsT=onesr,
                                 rhs=st["mb"][:], start=True, stop=True)
                st["bcsb"] = sm.tile([128, 4], F32, tag="bcsb", name="bcsb_t")

            def bcast1_evict(pair):
                st = PR[pair]
                nc.vector.tensor_copy(st["bcsb"][:], st["misc"][:, 8:12])

            def cprime(pair):
                """c' = cA - muL*cB  (cB host const)."""
                st = PR[pair]
                cpr = sm.tile([1, 2], F32, tag="cpr", name="cpr_t")
                st["cpr"] = cpr
                nc.vector.tensor_scalar_mul(cpr[:], st["mb"][0:1, 0:2],
                                            st["cB_const"])
                nc.vector.tensor_sub(cpr[:], st["cAsb"][:], cpr[:])

            def cA_evict(pair):
                st = PR[pair]
                st["cAsb"] = sm.tile([1, 2], F32, tag="cAsb", name="cAsb_t")
                nc.vector.tensor_copy(st["cAsb"][:], st["misc"][0:1, 14:16])

            def bcast2_mm(pair):
                st = PR[pair]
                nc.tensor.matmul(st["misc"][:, 12:14], lhsT=onesr,
                                 rhs=st["cpr"][:], start=True, stop=True)

            def scores_mm(pair, which, js):
                """which=0: sA = h0b^T q0 (cols 16:32); 1: sB = h0b^T arow (48:64)."""
                st = PR[pair]
                stp, h0b = st["stp"], st["h0b"]
                base = 16 if which == 0 else 48
                for j in js:
                    for s01 in range(2):
                        o = 64 * s01
                        rhs = (st["q0sb"][o:o + 64, :] if which == 0
                               else arow2[o:o + 64, :])
                        nc.tensor.matmul(
                            stp[:, base + 2 * j + s01:base + 2 * j + s01 + 1],
                            lhsT=h0b[o:o + 64, 128 * j:128 * j + 128],
                            rhs=rhs, start=True, stop=True)

            def stats_dve(pair):
                """mean, rstd tiles from stp sums."""
                st = PR[pair]
                stp = st["stp"]
                st["mean"] = sm.tile([128, 16], F32, tag="mean", name="mean_t")
                st["rstd"] = sm.tile([128, 16], F32, tag="rstd", name="rstd_t")
                tmp = sm.tile([128, 16], F32, tag="tmpa", name="tmpa_t")
                nc.vector.tensor_scalar_mul(st["mean"][:], stp[:, 0:16], 1.0 / D)
                nc.vector.tensor_scalar_mul(tmp[:], stp[:, 32:48], 1.0 / D)
                t2 = sm.tile([128, 16], F32, tag="tmpb", name="tmpb_t")
                nc.vector.tensor_mul(t2[:], st["mean"][:], st["mean"][:])
                nc.vector.tensor_sub(tmp[:], tmp[:], t2[:])
                nc.scalar.activation(tmp[:], tmp[:], AF.Ln, bias=epsb[:])
                nc.scalar.activation(st["rstd"][:], tmp[:], AF.Exp, scale=-0.5)

            def assemble_exp(pair):
                """sc = (sA - muL*sB - mu*c') * rstd * rL / 8 ; exps = exp(sc)."""
                st = PR[pair]
                stp, misc = st["stp"], st["misc"]
                bc, mean, rstd = st["bcsb"], st["mean"], st["rstd"]
                t1 = sm.tile([128, 16], F32, tag="t1", name="t1_t")
                v3 = lambda ap: ap.rearrange("p (j t) -> p j t", t=2)
                # t1 = sB * muL
                nc.vector.tensor_tensor(
                    v3(t1[:]), v3(stp[:, 48:64]),
                    bc[:, 0:2][:, None].to_broadcast([128, NJ, 2]), op=OP.mult)
                nc.vector.tensor_sub(t1[:], stp[:, 16:32], t1[:])
                t3 = sm.tile([128, 16], F32, tag="t3", name="t3_t")
                nc.vector.tensor_tensor(
                    v3(t3[:]), v3(mean[:]),
                    misc[:, 12:14][:, None].to_broadcast([128, NJ, 2]), op=OP.mult)
                nc.vector.tensor_sub(t1[:], t1[:], t3[:])
                # r8 = rstd * rL (the 1/8 is folded into q0sb/arow)
                r8 = sm.tile([128, 16], F32, tag="r8", name="r8_t")
                nc.vector.tensor_tensor(
                    v3(r8[:]), v3(rstd[:]),
                    bc[:, 2:4][:, None].to_broadcast([128, NJ, 2]), op=OP.mult)
                nc.vector.tensor_mul(t1[:], t1[:], r8[:])
                # eg = [exps | et | gt]; exps written inline at cols 0:16
                eg = sm.tile([128, 48], BF16, tag="eg", name="eg_t")
                st["eg"] = eg
                nc.scalar.activation(eg[:, 0:16], t1[:], AF.Exp)
                # zero padded token 1023 (j=7, partition 127)
                nc.vector.tensor_scalar(eg[:, 14:16], eg[:, 14:16],
                                        lastmf, None, op0=OP.mult)

            def etgt(pair):
                """et = exps*rstd (att weights), gt = et*mu (g correction)."""
                st = PR[pair]
                eg = st["eg"]
                nc.vector.tensor_mul(eg[:, 16:32], eg[:, 0:16], st["rstd"][:])
                nc.vector.tensor_mul(eg[:, 32:48], eg[:, 16:32], st["mean"][:])

            def zmm1(pair):
                st = PR[pair]
                nc.tensor.matmul(st["misc"][0:48, 16:17], lhsT=st["eg"][:],
                                 rhs=onesb, start=True, stop=True)

            def z_evict1(pair):
                st = PR[pair]
                st["z48"] = sm.tile([48, 1], F32, tag="z48", name="z48_t")
                nc.vector.tensor_copy(st["z48"][:], st["misc"][0:48, 16:17])

            def zmm2(pair):
                # -> [Z_A Z_B gs_A gs_B] at misc[0:1, 17:21]
                st = PR[pair]
                nc.tensor.matmul(st["misc"][0:1, 17:21], lhsT=st["z48"][:],
                                 rhs=g48, start=True, stop=True)

            def z_scalar(pair):
                st = PR[pair]
                z4 = sm.tile([1, 4], F32, tag="z4", name="z4_t")
                rg = sm.tile([1, 4], F32, tag="rg", name="rg_t")
                st["rg"] = rg
                nc.vector.tensor_copy(z4[:], st["misc"][0:1, 17:21])
                nc.vector.reciprocal(rg[0:1, 0:2], z4[0:1, 0:2])
                # raw g (un-normalized); the rz factor is applied once in oc
                nc.vector.tensor_copy(rg[0:1, 2:4], z4[0:1, 2:4])

            def bcastz_mm(pair):
                st = PR[pair]
                nc.tensor.matmul(st["misc"][0:64, 21:25], lhsT=onesr[0:1, 0:64],
                                 rhs=st["rg"][:], start=True, stop=True)
                st["bz"] = sm.tile([64, 4], F32, tag="bz", name="bz_t")

            def bcastz_evict(pair):
                st = PR[pair]
                nc.vector.tensor_copy(st["bz"][:], st["misc"][0:64, 21:25])

            def att_mm(pair, s01):
                st = PR[pair]
                misc = st["misc"]
                v0sb = st["v0sb%d" % s01]
                for j in range(NJ):
                    nc.tensor.matmul(
                        misc[0:64, 25 + s01:26 + s01],
                        lhsT=v0sb[:, 64 * j:64 * j + 64],
                        rhs=st["eg"][:, 16 + 2 * j + s01:17 + 2 * j + s01],
                        start=(j == 0), stop=(j == NJ - 1))

            def oc_proj(pair, s01):
                """oc = (attraw - g*svcol) * rz ; proj through ow.T."""
                st = PR[pair]
                misc, bz = st["misc"], st["bz"]
                oc = sm.tile([64, 1], F32, tag="oc", name="oc_t")
                nc.vector.tensor_scalar_mul(oc[:], svcol,
                                            bz[:, 2 + s01:3 + s01])
                nc.vector.tensor_sub(oc[:], misc[0:64, 25 + s01:26 + s01], oc[:])
                nc.vector.tensor_scalar(oc[:], oc[:], bz[:, s01:s01 + 1],
                                        None, op0=OP.mult)
                st["oc%d" % s01] = oc

            def proj_mm(pair, s01):
                st = PR[pair]
                nc.tensor.matmul(st["misc"][0:64, 27 + s01:28 + s01],
                                 lhsT=owt, rhs=st["oc%d" % s01][:],
                                 start=True, stop=True)

            def ha_evict(pair):
                st = PR[pair]
                c = 2 * pair
                nc.vector.tensor_copy(haf[0:D, c:c + 2], st["misc"][0:64, 27:29])
                nc.scalar.copy(hab[0:D, c:c + 2], st["misc"][0:64, 27:29])

            # ------- tail: per-pair head (overlaps other pair's attn) -------
            def tail_logits_mm(pair):
                st = PR[pair]
                c = 2 * pair
                nc.tensor.matmul(st["misc"][0:2, 29:37],
                                 lhsT=haf[0:D, c:c + 2],
                                 rhs=rwt, start=True, stop=True)

            def tail_gate(pair):
                """w4 = exp(logits) * (logits >= 4th largest); no Z (LN2-inv)."""
                st = PR[pair]
                misc = st["misc"]
                lg = sm.tile([2, 8], F32, tag="lg", name="lg_t")
                nc.vector.tensor_copy(lg[:], misc[0:2, 29:37])
                m8 = sm.tile([2, 8], F32, tag="m8", name="m8_t")
                nc.vector.max(m8[:], lg[:])
                msk = sm.tile([2, 8], F32, tag="msk", name="msk_t")
                nc.vector.tensor_scalar(msk[:], lg[:], m8[:, TOPK - 1:TOPK],
                                        None, op0=OP.is_ge)
                el = sm.tile([2, 8], F32, tag="el", name="el_t")
                nc.scalar.activation(el[:], misc[0:2, 29:37], AF.Exp)
                w4 = sm.tile([2, 8], F32, tag="w4", name="w4_t")
                st["w4"] = w4
                nc.vector.tensor_mul(w4[:], el[:], msk[:])

            def tail_eop_mm(pair):
                st = PR[pair]
                c = 2 * pair
                st["eop"] = ps.tile([128, 512], F32, tag="cps",
                                    name="eop_p%d" % pair, bufs=2)
                nc.tensor.matmul(st["eop"][0:2, :], lhsT=hab[:, c:c + 2],
                                 rhs=wexpb, start=True, stop=True)

            def tail_moe4(pair):
                st = PR[pair]
                prod = sm.tile([2, E * D], F32, tag="prod", name="prod_t")
                nc.vector.tensor_tensor(
                    prod[:].rearrange("p (e d) -> p e d", e=E),
                    st["eop"][0:2, :].rearrange("p (e d) -> p e d", e=E),
                    st["w4"][:].to_broadcast([2, E, D]), op=OP.mult)
                moe4 = sm.tile([2, D], F32, tag="moe4", name="moe4_t")
                st["moe4"] = moe4
                nc.vector.tensor_reduce(
                    moe4[:], prod[:].rearrange("p (e d) -> p d e", e=E),
                    mybir.AxisListType.X, OP.add)

            def tail_moeT_mm(pair):
                if "tl" not in TL:
                    TL["tl"] = ps.tile([128, 160], F32, tag="misc",
                                       name="tl_t", bufs=2)
                nc.tensor.transpose(TL["tl"][0:D, 2 * pair:2 * pair + 2],
                                    PR[pair]["moe4"][:], id4[0:2, 0:2])

            # ------- tail end: all 4 samples in one short chain -------
            def tail_end():
                tl = TL["tl"]
                moet = sm.tile([D, SPC], F32, tag="moet", name="moet_t")
                nc.vector.tensor_copy(moet[:], tl[0:D, 0:4])
                nc.tensor.matmul(tl[0:D, 4:8], lhsT=mowt, rhs=moet[:],
                                 start=True, stop=True)
                hm = sm.tile([D, SPC], F32, tag="hm", name="hm_t")
                nc.vector.tensor_copy(hm[:], tl[0:D, 4:8])
                hmsq = sm.tile([D, SPC], F32, tag="hmsq", name="hmsq_t")
                nc.scalar.activation(hmsq[:], hm[:], AF.Square)
                nc.tensor.matmul(tl[0:1, 8:12], lhsT=ones64, rhs=hm[:],
                                 start=True, stop=True)
                nc.tensor.matmul(tl[0:1, 12:16], lhsT=ones64, rhs=hmsq[:],
                                 start=True, stop=True)
                nc.tensor.matmul(tl[0:SPC, 32:128], lhsT=hm[:], rhs=outwt,
                                 start=True, stop=True)
                mur = sm.tile([1, 2 * SPC], F32, tag="mur", name="mur_t")
                nc.vector.tensor_scalar_mul(mur[0:1, 0:4], tl[0:1, 8:12],
                                            1.0 / D)
                ex = sm.tile([1, SPC], F32, tag="ex2", name="ex2_t")
                nc.vector.tensor_scalar_mul(ex[:], tl[0:1, 12:16], 1.0 / D)
                t2 = sm.tile([1, SPC], F32, tag="vr2", name="vr2_t")
                nc.vector.tensor_mul(t2[:], mur[0:1, 0:4], mur[0:1, 0:4])
                nc.vector.tensor_sub(t2[:], ex[:], t2[:])
                nc.scalar.activation(t2[:], t2[:], AF.Ln, bias=epsb[0:1, :])
                nc.scalar.activation(mur[0:1, 4:8], t2[:], AF.Exp, scale=-0.5)
                nc.tensor.matmul(tl[0:SPC, 16:17], lhsT=mur[0:1, 0:4],
                                 rhs=onesr[0:1, 0:1], start=True, stop=True)
                nc.tensor.matmul(tl[0:SPC, 17:18], lhsT=mur[0:1, 4:8],
                                 rhs=onesr[0:1, 0:1], start=True, stop=True)
                murT = sm.tile([SPC, 2], F32, tag="murT", name="murT_t")
                nc.vector.tensor_copy(murT[:], tl[0:SPC, 16:18])
                t = sm.tile([SPC, PRED], F32, tag="tout", name="tout_t")
                nc.vector.tensor_scalar(t[:], ocs4, murT[:, 0:1], None,
                                        op0=OP.mult)
                nc.vector.tensor_sub(t[:], tl[0:SPC, 32:128], t[:])
                outp = sm.tile([SPC, PRED], F32, tag="outp", name="outp_t")
                nc.scalar.activation(outp[:], t[:], AF.Copy,
                                     scale=murT[:, 1:2])
                nc.scalar.dma_start(Yout.ap(), outp[:])

            # store host const cB (sum of arow) placeholder; filled via closure
            # at prep time it is baked into the DVE immediate below.
            for pair in range(NPAIR):
                PR[pair]["cB_const"] = CB_SUM_AROW[0]

            # ================= emission schedule =================
            def frontA(pair):
                """m1-dependent attn work: j7 scalar path, q0, scores/v0 j4-7."""
                alloc_attn(pair)
                stats_mm(pair, [7])
                j7_path_a(pair)          # DVE copy + ext mm
                stats_mm(pair, [4, 5, 6])
                q0_mm(pair)
                q0_evict(pair)
                j7_path_b(pair)          # e4 copy + transpose mm
                v0_mm(pair, 0, [4, 5, 6, 7])
                v0_mm(pair, 1, [4, 5, 6, 7])
                j7_path_c(pair)          # scalar DVE/Act -> mb
                scores_mm(pair, 0, [4, 5, 6, 7])
                scores_mm(pair, 1, [4, 5, 6, 7])
                cA_mm(pair)
                cA_evict(pair)

            def frontB(pair):
                """m0-dependent attn work + broadcasts + stats pipeline."""
                stats_mm(pair, [0, 1, 2, 3])
                scores_mm(pair, 0, [0, 1, 2, 3])
                bcast1_mm(pair)
                bcast1_evict(pair)
                scores_mm(pair, 1, [0, 1, 2, 3])
                cprime(pair)
                v0_mm(pair, 0, [0, 1, 2, 3])
                v0_mm(pair, 1, [0, 1, 2, 3])
                bcast2_mm(pair)
                stats_dve(pair)
                v0_evict(pair, 0)
                v0_evict(pair, 1)

            def attn_mid(pair):
                assemble_exp(pair)
                etgt(pair)

            def attn_z1(pair):
                zmm1(pair)
                z_evict1(pair)

            def attn_z2(pair):
                zmm2(pair)
                z_scalar(pair)

            def attn_back(pair):
                bcastz_mm(pair)
                bcastz_evict(pair)
                att_mm(pair, 0)
                att_mm(pair, 1)
                oc_proj(pair, 0)
                oc_proj(pair, 1)

            def attn_proj(pair):
                proj_mm(pair, 0)
                proj_mm(pair, 1)
                ha_evict(pair)

            # --- pair 0 conv (m1 halves first so j7/scalar path starts early)
            conv_m(0, 0, 1)
            conv_m(0, 0, 0)
            conv_m(0, 1, 1)
            evict_m(0, 1)
            conv_m(0, 1, 0)
            evict_m(0, 0)
            frontA(0)
            # --- pair 1 conv interleaved with pair-0 attn back half ---
            conv_m(1, 0, 1)
            frontB(0)
            conv_m(1, 1, 1)
            evict_m(1, 1)
            attn_mid(0)
            attn_z1(0)
            frontA(1)
            conv_m(1, 0, 0)
            attn_z2(0)
            conv_m(1, 1, 0)
            evict_m(1, 0)
            attn_back(0)
            attn_proj(0)
            tail_logits_mm(0)
            tail_gate(0)
            frontB(1)
            tail_eop_mm(0)
            tail_moe4(0)
            attn_mid(1)
            attn_z1(1)
            tail_moeT_mm(0)
            attn_z2(1)
            attn_back(1)
            attn_proj(1)
            tail_logits_mm(1)
            tail_gate(1)
            tail_eop_mm(1)
            tail_moe4(1)
            tail_moeT_mm(1)
            tail_end()

    nc.compile()
    return nc


# cB = sum(arow) -- a host-side constant baked into the kernel IR. It is
# computed from the weights at prep time; since build happens after prep's
# first call, stash it in a module-level box.
CB_SUM_AROW = [0.0]

_NC_CACHE = {}


def _get_nc():
    if "nc" not in _NC_CACHE:
        _NC_CACHE["nc"] = build_nc()
    return _NC_CACHE["nc"]


def _prep_in_maps(inputs):
    f32 = np.float32
    X = np.ascontiguousarray(inputs["X"], f32)
    conv_w = np.asarray(inputs["conv_w"], f32)
    conv_b = np.asarray(inputs["conv_b"], f32)
    qw, kw, vw, ow = (np.asarray(inputs[k], f32) for k in ("qw", "kw", "vw", "ow"))
    expert_w = np.asarray(inputs["expert_w"], f32)
    expert_b = np.asarray(inputs["expert_b"], f32)
    router_w = np.asarray(inputs["router_w"], f32)
    moe_out_w = np.asarray(inputs["moe_out_w"], f32)
    out_w = np.asarray(inputs["out_w"], f32)

    nb = np.dtype(mybir.dt.np(BF16))

    # --- CB (bf16) ---
    CB = np.zeros((128, CBW), f32)
    Wc = conv_w.transpose(2, 1, 0).reshape(C * P, D)  # rows (p*64+c) -> k*128+r
    CB[:, CB_WSB:CB_WSB + 768] = np.ascontiguousarray(
        Wc.reshape(NCH, 128, D).transpose(1, 0, 2).reshape(128, NCH * D))
    A_T = qw.T @ kw                    # lhsT for q0 = A @ h0_last
    CB[:, CB_AT:CB_AT + D] = np.concatenate([A_T, A_T], axis=0)
    vwT = vw.T
    CB[:, CB_VWT:CB_VWT + D] = np.concatenate([vwT, vwT], axis=0)
    CB[0:64, CB_SEL] = 1.0
    CB[64:128, CB_SEL + 1] = 1.0
    arow = (kw.T @ qw.sum(1)) / 8.0    # A @ 1, with the 1/sqrt(D) folded in
    CB[:, CB_AROW] = np.concatenate([arow, arow], axis=0)
    CB[:, CB_ONE] = 1.0
    CB[:, CB_LASTM] = 1.0
    CB[127, CB_LASTM] = 0.0
    WexpE = np.concatenate(
        [expert_w.transpose(2, 0, 1).reshape(D, E * D),
         expert_b.reshape(1, E * D)], axis=0)
    CB[0:D + 1, CB_WEXP:CB_WEXP + E * D] = WexpE
    CB_SUM_AROW[0] = float(arow.sum())

    # --- CF (f32) ---
    CF = np.zeros((128, CFW), f32)
    pebT = (_pos_encoding_np(N, D) + conv_b[None, :]).T  # (64, N)
    CF[0:64, CF_PEBT:CF_PEBT + N] = pebT
    CF[64:128, CF_PEBT:CF_PEBT + N] = pebT
    CF[0:D, CF_OWT:CF_OWT + D] = ow.T
    CF[0:D, CF_MOWT:CF_MOWT + D] = moe_out_w.T
    CF[0:D, CF_OUTWT:CF_OUTWT + PRED] = out_w.T
    CF[0:D, CF_RWT:CF_RWT + E] = router_w.T
    CF[0:D, CF_SV] = vw.sum(1)
    CF[126, CF_OH] = 1.0
    CF[0:4, CF_ID4:CF_ID4 + 4] = np.eye(4, dtype=f32)
    # rows 0:16 (exps) -> Z_{A,B}; rows 16:32 (et) unused; 32:48 (gt) -> g
    G48 = np.zeros((48, 4), f32)
    for i in range(16):
        G48[i, i % 2] = 1.0
        G48[32 + i, 2 + (i % 2)] = 1.0
    CF[0:48, CF_G32:CF_G32 + 4] = G48
    CF[0:1, CF_ONESR:CF_ONESR + 128] = 1.0
    CF[0:D, CF_ONE64] = 1.0
    ocs = out_w.T.sum(0)               # (96,)
    CF[0:SPC, CF_OCS:CF_OCS + PRED] = np.stack([ocs] * SPC, axis=0)
    CF[:, CF_LASTM] = 1.0
    CF[127, CF_LASTM] = 0.0

    # --- Xd: host-deinterleaved even/odd columns, bf16 ---
    Xr = X.reshape(B, C, LH, 2)
    common = dict(
        CB=np.ascontiguousarray(CB).astype(nb),
        CF=np.ascontiguousarray(CF),
    )
    in_maps = []
    for c in range(NCORES):
        m = dict(common)
        xr = Xr[c * SPC:(c + 1) * SPC]
        xd = np.concatenate([xr[..., 0], xr[..., 1]], axis=1)  # (SPC, 128, LH)
        m["Xd"] = np.ascontiguousarray(xd).astype(nb)
        in_maps.append(m)
    return in_maps


def kernel(**inputs) -> np.ndarray:
    in_maps = _prep_in_maps(inputs)
    nc = _get_nc()
    res = run_bass_kernel_spmd(nc, in_maps, core_ids=list(range(NCORES)))
    out = np.concatenate([res.results[c]["Yout"] for c in range(NCORES)], axis=0)
    return out.astype(np.float32)



# revision 62
# speedup vs baseline: 1.3351x; 1.3351x over previous
"""Trainium2 Bass kernel for nn_Decoder_22703197127089 (moe_routing).

Only the last token survives to the output (h[:, -1, :] is taken after the
MoE block), so per sample we need: conv patch-embed for all 1023 tokens
(feeds K/V), folded-LN attention row for the last token, MoE + LN2 + head
for one token.

v3 design notes (on top of the v2 folded-LN / deinterleaved-X baseline):
  - The conv runs in fp8e4 DoubleRow mode (0.5 cyc/row): X is shipped as
    fp8 hi (Xh) plus a 64x-scaled fp8 residual (Xl); conv weights ride as
    fp8e4(64*W) (pairs Xh), fp8e4(W) (pairs Xl), and an e5m2 second-order
    residual fp8e5(64W - fp8(64W)) (pairs Xh).  psum accumulates 64*h0;
    the 64x scale cancels through the folded-LN score algebra, so only
    pebt (x64) and the LN eps (x64^2) change.
  - The whole attention/MoE back-end is batched 4-wide: one shared stats
    pipeline, one z-reduction, one oc/proj, one MoE tail for all 4
    samples, cutting the exposed end-of-kernel serial chain in half.
  - moe_out_w is folded into the expert weights host-side.
  - z-reduction uses a single colsum matmul + grouped tensor_reduce;
    scalar_tensor_tensor fuses the mean/var and c' arithmetic.

Sharding: data-parallel over batch B=32 across 8 cores (4 samples/core),
host gathers (4, 96) per-core outputs.
"""

import numpy as np

import concourse.bass as bass
import concourse.mybir as mybir
import concourse.tile as tile
from concourse import bacc
from concourse.bass_utils import run_bass_kernel_spmd

F32 = mybir.dt.float32
BF16 = mybir.dt.bfloat16
FP16 = mybir.dt.float16
FP8E4 = mybir.dt.float8e4
FP8E5 = mybir.dt.float8e5
DR = mybir.MatmulPerfMode.DoubleRow
AF = mybir.ActivationFunctionType
OP = mybir.AluOpType
AX = mybir.AxisListType

B, C, L = 32, 64, 12288
D = 64
E = 8
TOPK = 4
P, S = 24, 12
PRED = 96
N = (L - P) // S + 1  # 1023
NCORES = 8
SPC = B // NCORES     # 4 samples per core
NPAIR = SPC // 2      # 2
NCH = (C * P) // 128  # 12 contraction chunks of K=128
NDR = NCH // 2        # 6 DoubleRow chunk-pairs (K=256 each)
NT = 1024             # padded token dim (col 1023 zeroed)
NJ = 8                # 128-token chunks
EPS = 1e-5
EPS64 = EPS * 64.0 * 64.0   # LN eps at the 64x h0 scale
LH = 6144             # deinterleaved X columns
XSPLIT = 3072         # X half-DMA split: m2/m3 read only [3072:6144]; m0/m1
                      # read [0:3078] (m1's last patch peeks 6 cols into h1,
                      # which always lands first)
MCS = (256, 256, 256, 255)  # conv m-chunk sizes (patches); DoubleRow rhs
                            # free size 2*nn must stay <= 512

# ---- CB (bf16) column map ----
CB_AT = 0             # A^T = qw.T @ kw, doubled rows        (64)
CB_VWT = 64           # vw.T doubled                          (64)
CB_SEL = 128          # selab                                 (2)
CB_AROW = 130         # arow doubled                          (1)
CB_ONE = 131          # ones column                           (1)
CB_WEXP = 136         # moe_out-folded experts (rows 0:65)    (512)
CBW = 648

# ---- C8 (fp8e4) column map: conv weights, 64x and 1x scale ----
C8_W8 = 0             # fp8e4(64*W) chunks                    (768)
C8_WD = 768           # fp8e4(W) chunks (pairs the Xl stream) (768)
C8W = 1536

# C5 (fp8e5): Wr8 = fp8e5(64*W - W8) chunks                   (768)
C5W = 768

# PB (fp16): 64*(pe + conv_b).T doubled rows, col 1023 zero   (1024)
PBW = 1024

# ---- CF (f32) column map ----
CF_OWT = 0            # ow.T rows 0:64                        (64)
CF_OUTWT = 64         # out_w.T rows 0:64                     (96)
CF_RWT = 160          # router_w.T rows 0:64                  (8)
CF_SV = 168           # vw.sum(1) rows 0:64                   (1)
CF_OH = 169           # onehot at partition 126               (1)
CF_ID4 = 170          # eye(4) rows 0:4                       (4)
CF_ID8 = 174          # eye(8) rows 0:8                       (8)
CF_ONESR = 182        # ones row 0                            (128)
CF_ONE64 = 310        # ones rows 0:64                        (1)
CF_OCS = 311          # out_w.T colsums rows 0:4              (96)
CF_LASTM = 407        # ones, 0 at partition 127              (1)
CFW = 408

# ---- stp4 (psum, shared) column map: 4-wide per-token stats ----
# sums 0:32 (col 4j+s), sumsq 32:64, sA 64:96, sB 96:128

# ---- misc4 (psum, shared) column map ----
MQ0 = 0               # q0 per sample                         (4)
MJ7R = 4              # j7 row [1,8]                          (8)
MB1 = 12              # bcast1: muL 12:16, rL 16:20           (8)
MB2 = 20              # bcast2: c'                            (4)
MCA = 24              # cA [1,4]                              (4)
MZR = 28              # zrow [1,64]                           (64)
MBZ = 92              # bcastz: rz 92:96, gz 96:100           (8)
MATT = 100            # attraw [64,4]                         (4)
MHA = 104             # proj ha [64,4]                        (4)
MLG = 108             # logits [4,8]                          (8)
MMT = 120             # moeT [64,4]                           (4)
MSM = 124             # LN2 sums [1,8]                        (8)
MMU = 132             # mur bcast [4,2]                       (2)
MHR = 160             # head raw [4,96]                       (96)


def _pos_encoding_np(n, d):
    pos = np.arange(n, dtype=np.float32)[:, None]
    div = np.exp(np.arange(0, d, 2, dtype=np.float32)
                 * (np.float32(-np.log(np.float32(10000.0))) / np.float32(d)))
    pe = np.zeros((n, d), np.float32)
    pe[:, 0::2] = np.sin(pos * div)
    pe[:, 1::2] = np.cos(pos * div)
    return pe


def build_nc():
    nc = bacc.Bacc("TRN2", target_bir_lowering=False, debug=False,
                   num_devices=NCORES)

    Xht = nc.dram_tensor("Xh", [SPC, 128, LH], FP8E4, kind="ExternalInput")
    XlLt = nc.dram_tensor("XlL", [128, SPC * 12], FP8E4, kind="ExternalInput")
    CBt = nc.dram_tensor("CB", [128, CBW], BF16, kind="ExternalInput")
    C8t = nc.dram_tensor("C8", [128, C8W], FP8E4, kind="ExternalInput")
    C5t = nc.dram_tensor("C5", [128, C5W], FP8E5, kind="ExternalInput")
    PBt = nc.dram_tensor("PB", [128, PBW], FP16, kind="ExternalInput")
    CFt = nc.dram_tensor("CF", [128, CFW], F32, kind="ExternalInput")
    Yout = nc.dram_tensor("Yout", [SPC, PRED], F32, kind="ExternalOutput")

    with tile.TileContext(nc) as tc:
        with (
            tc.tile_pool(name="const", bufs=1) as pc,
            tc.tile_pool(name="xp", bufs=4) as xp,
            tc.tile_pool(name="hp", bufs=2) as hp,
            tc.tile_pool(name="sm", bufs=2) as sm,
            tc.tile_pool(name="vp", bufs=2) as vp,
            tc.tile_pool(name="ps", bufs=1, space="PSUM") as ps,
        ):
            # ---------------- constants / warmup ----------------
            cb = pc.tile([128, CBW], BF16, tag="cb")
            c8 = pc.tile([128, C8W], FP8E4, tag="c8")
            c5 = pc.tile([128, C5W], FP8E5, tag="c5")
            pb = pc.tile([128, PBW], FP16, tag="pb")
            cf = pc.tile([128, CFW], F32, tag="cf")
            junk = pc.tile([128, 512], BF16, tag="junk")
            epsb = pc.tile([128, 1], F32, tag="epsb")
            eps64b = pc.tile([128, 1], F32, tag="eps64b")

            xht = []
            for s in range(SPC):
                xht.append(xp.tile([128, LH], FP8E4, tag="xh", name="xh%d" % s))
            xll = xp.tile([128, SPC * 12], FP8E4, tag="xll", name="xll")

            def xdma(s, half):
                t = xht[s]
                src = Xht.ap()[s]
                if half == 0:
                    nc.sync.dma_start(t[:, 0:XSPLIT], src[:, 0:XSPLIT])
                else:
                    nc.sync.dma_start(t[:, XSPLIT:LH], src[:, XSPLIT:LH])

            # DMA order: conv weights first, then per-sample halves, upper
            # (h1) halves before lower (m2/m3 hold the last token + j7
            # stats); small consts ride between X transfers; the expert
            # weights (only needed by the tail) go last.
            xdma(0, 1)
            nc.sync.dma_start(c8[:], C8t.ap())
            nc.sync.dma_start(c5[:], C5t.ap())
            nc.sync.dma_start(xll[:], XlLt.ap())
            xdma(0, 0)
            nc.sync.dma_start(pb[:], PBt.ap())
            nc.sync.dma_start(cb[:, 0:CB_WEXP], CBt.ap()[:, 0:CB_WEXP])
            xdma(1, 1)
            xdma(1, 0)
            nc.sync.dma_start(cf[:], CFt.ap())
            xdma(2, 1)
            xdma(3, 1)
            xdma(2, 0)
            xdma(3, 0)
            nc.sync.dma_start(cb[:, CB_WEXP:CBW], CBt.ap()[:, CB_WEXP:CBW])

            nc.vector.memset(junk[:], 0.25)
            nc.vector.memset(epsb[:], EPS)
            nc.vector.memset(eps64b[:], EPS64)

            # Pre-load the one act-function set covering Square/Exp/Ln/Copy
            from concourse.hw_specs import get_activation_tables
            _set_id = list(get_activation_tables(nc.m.arch)).index(
                "natural_log_exp_and_others")
            nc.scalar.add_instruction(mybir.InstLoadActFuncSet(
                name=nc.get_next_instruction_name(), ins=[], outs=[],
                act_func_set_id=_set_id))

            w8v = c8[:, C8_W8:C8_W8 + 768].rearrange("p (k d) -> p k d", k=NCH)
            wdv = c8[:, C8_WD:C8_WD + 768].rearrange("p (k d) -> p k d", k=NCH)
            wrv = c5[:, 0:768].rearrange("p (k d) -> p k d", k=NCH)
            at2 = cb[:, CB_AT:CB_AT + 64]
            vwt2 = cb[:, CB_VWT:CB_VWT + 64]
            selab = cb[:, CB_SEL:CB_SEL + 2]
            arow2 = cb[:, CB_AROW:CB_AROW + 1]
            onesb = cb[:, CB_ONE:CB_ONE + 1]
            lastmf = cf[:, CF_LASTM:CF_LASTM + 1]
            wexpb = cb[0:D + 1, CB_WEXP:CB_WEXP + E * D]
            owt = cf[0:D, CF_OWT:CF_OWT + D]
            outwt = cf[0:D, CF_OUTWT:CF_OUTWT + PRED]
            rwt = cf[0:D, CF_RWT:CF_RWT + E]
            svcol = cf[0:D, CF_SV:CF_SV + 1]
            onehotf = cf[:, CF_OH:CF_OH + 1]
            id4 = cf[0:4, CF_ID4:CF_ID4 + 4]
            id8 = cf[0:8, CF_ID8:CF_ID8 + 8]
            onesr = cf[0:1, CF_ONESR:CF_ONESR + 128]
            ones64 = cf[0:D, CF_ONE64:CF_ONE64 + 1]
            ocs4 = cf[0:SPC, CF_OCS:CF_OCS + PRED]

            # PE warmup: keep the tensor engine busy during initial DMA so
            # real matmuls dispatch at full p-state.
            jp = ps.tile([128, 512], F32, tag="v0p", name="junkp", bufs=2)
            jp2 = ps.tile([128, 512], F32, tag="v0p", name="junkp2", bufs=2)
            for i in range(7):
                t = jp if i % 2 == 0 else jp2
                nc.tensor.matmul(t[:, 0:512], lhsT=junk[:, 0:128],
                                 rhs=junk[:, 0:512], start=True, stop=True)

            # Shared attention-output tiles (row 64 = 1.0 feeds expert bias)
            haf = pc.tile([D + 1, SPC], F32, tag="haf")
            hab = pc.tile([D + 1, SPC], BF16, tag="hab")
            nc.vector.memset(haf[D:D + 1, :], 1.0)
            nc.vector.memset(hab[D:D + 1, :], 1.0)

            # Shared 4-wide attention state
            stp4 = ps.tile([128, 128], F32, tag="stp", name="stp4")
            misc4 = ps.tile([128, 256], F32, tag="misc", name="misc4")
            eg4 = sm.tile([128, 96], BF16, tag="eg", name="eg4")
            PR = [dict() for _ in range(NPAIR)]
            SH = {}

            # ---------------- emission helpers ----------------
            def conv_m(pair, s01, mi):
                """12 DoubleRow accumulation matmuls for sample (2*pair+s01),
                m-chunk mi: 6 chunk-pairs x {Xh*W8, Xh*Wr8}; psum holds
                64*h0.  m3 adds 6 narrow Xl*Wd8 DRs correcting the last
                token (col 510).  m-chunks {0,1} share cpsL, {2,3} cpsH."""
                st = PR[pair]
                s = 2 * pair + s01
                n0 = 256 * mi
                nn = MCS[mi]
                # DoubleRow dst partitions must start at 0 (s3d3 ISA rule),
                # so every sample gets its own base-0 psum tile.
                key = "cps%d_%d" % (mi, s01)
                if key not in st:
                    st[key] = ps.tile([64, 256], F32, tag="cps",
                                      name="cps_p%d_%s" % (pair, key), bufs=3)
                cps = st[key]
                # [128, 6(r), 1024(n)]: dim1 = the DoubleRow k-tile pair
                xvh = xht[s][:].rearrange("p (n s) -> p s n", s=6)
                mms = ([(xvh, w8v, j) for j in range(NDR)]
                       + [(xvh, wrv, j) for j in range(NDR)])
                last = mi != 3
                for i, (xv, wv, j) in enumerate(mms):
                    q, r = divmod(2 * j, 6)
                    nc.tensor.matmul(
                        cps[0:64, 0:nn],
                        lhsT=wv[:, 2 * j:2 * j + 2, :],
                        rhs=xv[:, r:r + 2, n0 + q:n0 + q + nn],
                        start=(i == 0), stop=(last and i == len(mms) - 1),
                        perf_mode=DR)
                if mi == 3:
                    # last-token (1022 -> psum col 254) fp8-residual fixup:
                    # 12 plain single-column fp8 matmuls, one per K-chunk
                    xlv = xll[:].rearrange("p (s q r) -> p s q r",
                                           s=SPC, q=2, r=6)
                    for k in range(NCH):
                        q, r = divmod(k, 6)
                        nc.tensor.matmul(
                            cps[0:64, 254:255],
                            lhsT=wdv[:, k, :],
                            rhs=xlv[:, s, q, r:r + 1],
                            start=False, stop=(k == NCH - 1),
                            skip_group_check=True)

            def evict_m(pair, s01, mi):
                """psum + pebt64 -> h0b (bf16, 64x scale) on DVE; Square ->
                sq on Act.  Per-(sample, m-chunk) granularity: base-0 tiles
                keep DVE/Act lanes aligned with the base-0 conv psum."""
                st = PR[pair]
                hk, qk = "h0b%d" % s01, "sq%d" % s01
                if hk not in st:
                    st[hk] = hp.tile([64, NT], BF16, tag="h0b",
                                     name="h0b_p%d_s%d" % (pair, s01))
                    st[qk] = hp.tile([64, NT], BF16, tag="sq",
                                     name="sq_p%d_s%d" % (pair, s01))
                    nc.vector.memset(st[hk][:, N:NT], 0.0)
                    nc.vector.memset(st[qk][:, N:NT], 0.0)
                n0 = 256 * mi
                nn = MCS[mi]
                cps = st["cps%d_%d" % (mi, s01)]
                nc.vector.tensor_add(st[hk][:, n0:n0 + nn],
                                     cps[0:64, 0:nn], pb[0:64, n0:n0 + nn])
                nc.scalar.activation(st[qk][:, n0:n0 + nn],
                                     st[hk][:, n0:n0 + nn], AF.Square)

            def stats_mm(pair, js):
                """Per-token sums / sum-of-squares into stp4 col 4j+s."""
                st = PR[pair]
                for j in js:
                    for s01 in range(2):
                        c = 4 * j + 2 * pair + s01
                        h0b, sq = st["h0b%d" % s01], st["sq%d" % s01]
                        nc.tensor.matmul(stp4[:, c:c + 1],
                                         lhsT=h0b[:, 128 * j:128 * j + 128],
                                         rhs=onesb[0:64, :],
                                         start=True, stop=True)
                        nc.tensor.matmul(stp4[:, 32 + c:32 + c + 1],
                                         lhsT=sq[:, 128 * j:128 * j + 128],
                                         rhs=onesb[0:64, :],
                                         start=True, stop=True)

            def q0_mm(pair):
                for s01 in range(2):
                    s = 2 * pair + s01
                    nc.tensor.matmul(misc4[0:64, MQ0 + s:MQ0 + s + 1],
                                     lhsT=at2[0:64, :],
                                     rhs=PR[pair]["h0b%d" % s01][:, N - 1:N],
                                     start=True, stop=True)

            def q0_evict(pair):
                # 1/sqrt(D)=1/8 score scale folded here (and into host arow)
                st = PR[pair]
                for s01 in range(2):
                    s = 2 * pair + s01
                    t = sm.tile([64, 1], BF16, tag="q0sb",
                                name="q0sb_s%d" % s)
                    st["q0sb%d" % s01] = t
                    nc.vector.tensor_scalar_mul(
                        t[:], misc4[0:64, MQ0 + s:MQ0 + s + 1], 0.125)

            def cA_mm(pair):
                # cA = sum(q0) per sample -> misc4[0:1, MCA+s]
                for s01 in range(2):
                    s = 2 * pair + s01
                    nc.tensor.matmul(misc4[0:1, MCA + s:MCA + s + 1],
                                     lhsT=PR[pair]["q0sb%d" % s01][:],
                                     rhs=onesb[0:64, :],
                                     start=True, stop=True)

            def v0_mm(pair, s01, js):
                st = PR[pair]
                key = "v0p%d" % s01
                if key not in st:
                    st[key] = ps.tile([128, 512], F32, tag="v0p",
                                      name="v0_p%d_s%d" % (pair, s01), bufs=2)
                v0p = st[key]
                h0b = st["h0b%d" % s01]
                for j in js:
                    nc.tensor.matmul(v0p[:, 64 * j:64 * j + 64],
                                     lhsT=h0b[:, 128 * j:128 * j + 128],
                                     rhs=vwt2[0:64, :],
                                     start=True, stop=True)

            def v0_evict(pair, s01, half=None):
                """psum->SBUF staging for the attention values.  Pair-0 rides
                the idle Pool engine; pair-1 (latency-critical) is copied in
                hi/lo halves on DVE/Act in their idle windows."""
                st = PR[pair]
                key = "v0sb%d" % s01
                if key not in st:
                    st[key] = vp.tile([128, 512], BF16, tag="v0sb",
                                      name="v0sb_t")
                if half is None:
                    # GPSIMD cannot touch PSUM; split pair-0's copies over
                    # the DVE/Act queues (off the critical window).
                    if s01 == 0:
                        nc.scalar.copy(st[key][:], st["v0p%d" % s01][:])
                    else:
                        nc.vector.tensor_copy(st[key][:], st["v0p%d" % s01][:])
                    return
                c0, c1 = (256, 512) if half else (0, 256)
                if s01 == 0:
                    nc.vector.tensor_copy(st[key][:, c0:c1],
                                          st["v0p%d" % s01][:, c0:c1])
                else:
                    nc.scalar.copy(st[key][:, c0:c1],
                                   st["v0p%d" % s01][:, c0:c1])

            def scores_mm(pair, which, js):
                """which=0: sA = h0b^T q0 (stp4 64+); 1: sB = h0b^T arow (96+)."""
                st = PR[pair]
                base = 64 if which == 0 else 96
                for j in js:
                    for s01 in range(2):
                        c = base + 4 * j + 2 * pair + s01
                        h0b = st["h0b%d" % s01]
                        rhs = (st["q0sb%d" % s01][:] if which == 0
                               else arow2[0:64, :])
                        nc.tensor.matmul(
                            stp4[:, c:c + 1],
                            lhsT=h0b[:, 128 * j:128 * j + 128],
                            rhs=rhs, start=True, stop=True)

            # ------- 4-wide mid: last-token stats for all samples -------
            # Split a/b/c so the PE pieces interleave with pair-1 conv groups
            # without head-of-line blocking them.
            def j7_stats_mm(pair):
                """Row-form sum / sumsq of h0b col 1022 (K=64, M=1 matmuls
                with the h0b column itself as rhs for the square)."""
                for s01 in range(2):
                    s = 2 * pair + s01
                    col = PR[pair]["h0b%d" % s01][:, N - 1:N]
                    nc.tensor.matmul(misc4[0:1, MJ7R + s:MJ7R + s + 1],
                                     lhsT=col, rhs=onesb[0:64, :],
                                     start=True, stop=True)
                    nc.tensor.matmul(misc4[0:1, MJ7R + 4 + s:MJ7R + 5 + s],
                                     lhsT=col, rhs=col,
                                     start=True, stop=True)

            def j7_mid_a():
                """j7 raw row stats -> mb4 = [muL x4 | rL x4]; c'."""
                mb4 = sm.tile([1, 8], F32, tag="mb", name="mb4_t")
                SH["mb4"] = mb4
                t2j = sm.tile([1, 4], F32, tag="t2j", name="t2j_t")
                raw_s = misc4[0:1, MJ7R:MJ7R + 4]
                raw_q = misc4[0:1, MJ7R + 4:MJ7R + 8]
                nc.vector.tensor_scalar_mul(mb4[0:1, 0:4], raw_s, 1.0 / D)
                nc.vector.tensor_mul(t2j[:], mb4[0:1, 0:4], mb4[0:1, 0:4])
                tvj = sm.tile([1, 4], F32, tag="tvj", name="tvj_t")
                nc.vector.tensor_scalar_mul(tvj[:], raw_q, 1.0 / D)
                nc.vector.tensor_sub(t2j[:], tvj[:], t2j[:])
                nc.scalar.activation(t2j[:], t2j[:], AF.Ln, bias=eps64b[0:1, :])
                nc.scalar.activation(mb4[0:1, 4:8], t2j[:], AF.Exp, scale=-0.5)
                # c' = cA - muL*cB (the bcast rides in j7_mid_c)
                cAsb = sm.tile([1, 4], F32, tag="cAsb", name="cAsb_t")
                nc.vector.tensor_copy(cAsb[:], misc4[0:1, MCA:MCA + 4])
                cpr4 = sm.tile([1, 4], F32, tag="cpr", name="cpr4_t")
                SH["cpr4"] = cpr4
                nc.vector.tensor_scalar_mul(cpr4[:], SH["mb4"][0:1, 0:4],
                                            -CB_SUM_AROW[0])
                nc.vector.tensor_add(cpr4[:], cpr4[:], cAsb[:])

            def j7_mid_b():
                # broadcast muL/rL to 128 partitions
                nc.tensor.matmul(misc4[:, MB1:MB1 + 8], lhsT=onesr,
                                 rhs=SH["mb4"][:], start=True, stop=True)
                bcsb4 = sm.tile([128, 8], F32, tag="bcsb", name="bcsb4_t")
                SH["bcsb4"] = bcsb4
                nc.vector.tensor_copy(bcsb4[:], misc4[:, MB1:MB1 + 8])

            def j7_mid_c():
                nc.tensor.matmul(misc4[:, MB2:MB2 + 4], lhsT=onesr,
                                 rhs=SH["cpr4"][:], start=True, stop=True)

            # ------- 4-wide chain: stats -> exps -> z -> att -> proj -------
            def chain4():
                mean4 = sm.tile([128, 32], F32, tag="mean", name="mean4_t")
                rstd4 = sm.tile([128, 32], F32, tag="rstd", name="rstd4_t")
                t2a = sm.tile([128, 32], F32, tag="tmpa", name="t2a_t")
                nc.vector.tensor_scalar_mul(mean4[:], stp4[:, 0:32], 1.0 / D)
                tva = sm.tile([128, 32], F32, tag="tva", name="tva_t")
                nc.vector.tensor_scalar_mul(tva[:], stp4[:, 32:64], 1.0 / D)
                nc.vector.tensor_mul(t2a[:], mean4[:], mean4[:])
                nc.vector.tensor_sub(t2a[:], tva[:], t2a[:])
                nc.scalar.activation(t2a[:], t2a[:], AF.Ln, bias=eps64b[:])
                nc.scalar.activation(rstd4[:], t2a[:], AF.Exp, scale=-0.5)

                # sc = (sA - muL*sB - mu*c') * rstd * rL ; exps = exp(sc).
                # t3 rides on DVE during the Ln/Exp window; everything stays
                # on DVE to avoid cross-engine sem hops on the serial chain.
                bc = SH["bcsb4"]
                v4 = lambda ap: ap.rearrange("p (j s) -> p j s", s=4)
                t1 = sm.tile([128, 32], F32, tag="t1", name="t1_t")
                t3 = sm.tile([128, 32], F32, tag="t3", name="t3_t")
                nc.vector.tensor_tensor(
                    v4(t3[:]), v4(mean4[:]),
                    misc4[:, MB2:MB2 + 4][:, None].to_broadcast([128, NJ, 4]),
                    op=OP.mult)
                nc.vector.tensor_tensor(
                    v4(t1[:]), v4(stp4[:, 96:128]),
                    bc[:, 0:4][:, None].to_broadcast([128, NJ, 4]), op=OP.mult)
                nc.vector.tensor_sub(t1[:], stp4[:, 64:96], t1[:])
                nc.vector.tensor_sub(t1[:], t1[:], t3[:])
                r8 = sm.tile([128, 32], F32, tag="r8", name="r8_t")
                nc.vector.tensor_tensor(
                    v4(r8[:]), v4(rstd4[:]),
                    bc[:, 4:8][:, None].to_broadcast([128, NJ, 4]), op=OP.mult)
                nc.vector.tensor_mul(t1[:], t1[:], r8[:])
                # eg4 = [exps 0:32 | gt 32:64 | et 64:96].  The padded token
                # 1023 yields sc=0 -> exp=1 exactly; its +1 in Z is removed
                # below and it contributes nothing to att (v0 col is 0) or
                # gs (mean is 0), so no mask op is needed.
                nc.scalar.activation(eg4[:, 0:32], t1[:], AF.Exp)
                nc.vector.tensor_mul(eg4[:, 64:96], eg4[:, 0:32], rstd4[:])
                nc.vector.tensor_mul(eg4[:, 32:64], eg4[:, 64:96], mean4[:])

                # z-reduction: colsums -> grouped j-reduce -> rz, gz
                nc.tensor.matmul(
                    misc4[0:1, MZR:MZR + 64], lhsT=onesb,
                    rhs=eg4[:, 0:64].rearrange("p (g j s) -> p g s j",
                                               g=2, j=NJ, s=4),
                    start=True, stop=True)
                zg = sm.tile([1, 8], F32, tag="zg", name="zg_t")
                nc.vector.tensor_reduce(
                    zg[:],
                    misc4[0:1, MZR:MZR + 64].rearrange("p (a j) -> p a j", a=8),
                    AX.X, OP.add)
                # Z carries the pad token's exp(0)=1: subtract it here
                nc.vector.tensor_scalar_add(zg[0:1, 0:4], zg[0:1, 0:4], -1.0)
                rg4 = sm.tile([1, 8], F32, tag="rg", name="rg4_t")
                nc.vector.reciprocal(rg4[0:1, 0:4], zg[0:1, 0:4])
                nc.vector.tensor_mul(rg4[0:1, 4:8], zg[0:1, 4:8],
                                     rg4[0:1, 0:4])
                nc.tensor.matmul(misc4[0:64, MBZ:MBZ + 8],
                                 lhsT=onesr[0:1, 0:64], rhs=rg4[:],
                                 start=True, stop=True)
                bz = sm.tile([64, 8], F32, tag="bz", name="bz4_t")
                nc.vector.tensor_copy(bz[:], misc4[0:64, MBZ:MBZ + 8])

                # attraw per sample
                for s in range(SPC):
                    pair, s01 = divmod(s, 2)
                    v0sb = PR[pair]["v0sb%d" % s01]
                    for j in range(NJ):
                        nc.tensor.matmul(
                            misc4[0:64, MATT + s:MATT + s + 1],
                            lhsT=v0sb[:, 64 * j:64 * j + 64],
                            rhs=eg4[:, 64 + 4 * j + s:65 + 4 * j + s],
                            start=(j == 0), stop=(j == NJ - 1))

                # oc = attraw*rz - sv*gz ; project through ow.T
                svz = sm.tile([64, 4], F32, tag="svz", name="svz_t")
                nc.gpsimd.tensor_tensor(svz[:], svcol.to_broadcast([D, SPC]),
                                        bz[:, 4:8], op=OP.mult)
                tt4 = sm.tile([64, 4], F32, tag="tt4", name="tt4_t")
                nc.vector.tensor_tensor(tt4[:], misc4[0:64, MATT:MATT + 4],
                                        bz[:, 0:4], op=OP.mult)
                oc2 = sm.tile([64, 4], F32, tag="oc", name="oc2_t")
                nc.vector.tensor_sub(oc2[:], tt4[:], svz[:])
                nc.tensor.matmul(misc4[0:64, MHA:MHA + 4], lhsT=owt,
                                 rhs=oc2[:], start=True, stop=True)
                nc.vector.tensor_copy(haf[0:D, :], misc4[0:64, MHA:MHA + 4])
                nc.scalar.copy(hab[0:D, :], misc4[0:64, MHA:MHA + 4])

            # ------- 4-wide tail: router/topk -> experts -> LN2 -> head -----
            def tail4():
                nc.tensor.matmul(misc4[0:SPC, MLG:MLG + 8], lhsT=haf[0:D, :],
                                 rhs=rwt, start=True, stop=True)
                lg = misc4[0:SPC, MLG:MLG + 8]
                m8 = sm.tile([SPC, 8], F32, tag="m8", name="m8_t")
                nc.vector.max(m8[:], lg)
                msk = sm.tile([SPC, 8], F32, tag="msk", name="msk_t")
                nc.vector.tensor_scalar(msk[:], lg, m8[:, TOPK - 1:TOPK],
                                        None, op0=OP.is_ge)
                el = sm.tile([SPC, 8], F32, tag="el", name="el_t")
                nc.scalar.activation(el[:], lg, AF.Exp)
                w4 = sm.tile([SPC, 8], F32, tag="w4", name="w4_t")
                nc.vector.tensor_mul(w4[:], el[:], msk[:])
                eop = ps.tile([128, 512], F32, tag="eop", name="eop_t")
                nc.tensor.matmul(eop[0:SPC, :], lhsT=hab[:], rhs=wexpb,
                                 start=True, stop=True)
                prod = sm.tile([SPC, E * D], F32, tag="prod", name="prod_t")
                moe4 = sm.tile([SPC, D], F32, tag="moe4", name="moe4_t")
                nc.vector.tensor_tensor(
                    prod[:].rearrange("p (e d) -> p e d", e=E),
                    eop[0:SPC, :].rearrange("p (e d) -> p e d", e=E),
                    w4[:].to_broadcast([SPC, E, D]), op=OP.mult)
                nc.vector.tensor_reduce(
                    moe4[:], prod[:].rearrange("p (e d) -> p d e", e=E),
                    AX.X, OP.add)
                # h2 (moe_out folded into experts) -> LN2 -> head; sumsq via
                # K=64 M=1 matmuls using the hm column as its own rhs.
                nc.tensor.transpose(misc4[0:D, MMT:MMT + 4], moe4[:],
                                    id4[0:SPC, 0:SPC])
                hm8 = sm.tile([D, 2 * SPC], F32, tag="hm", name="hm8_t")
                nc.vector.tensor_copy(hm8[:, 0:4], misc4[0:D, MMT:MMT + 4])
                nc.tensor.matmul(misc4[0:1, MSM:MSM + 4], lhsT=ones64,
                                 rhs=hm8[:, 0:4], start=True, stop=True)
                for s in range(SPC):
                    nc.tensor.matmul(misc4[0:1, MSM + 4 + s:MSM + 5 + s],
                                     lhsT=hm8[0:D, s:s + 1],
                                     rhs=hm8[0:D, s:s + 1],
                                     start=True, stop=True)
                nc.tensor.matmul(misc4[0:SPC, MHR:MHR + PRED],
                                 lhsT=hm8[:, 0:4], rhs=outwt,
                                 start=True, stop=True)
                mur = sm.tile([1, 8], F32, tag="mur", name="mur_t")
                raw_s = misc4[0:1, MSM:MSM + 4]
                raw_q = misc4[0:1, MSM + 4:MSM + 8]
                nc.vector.tensor_scalar_mul(mur[0:1, 0:4], raw_s, 1.0 / D)
                t2t = sm.tile([1, 4], F32, tag="t2t", name="t2t_t")
                tvt = sm.tile([1, 4], F32, tag="tvt", name="tvt_t")
                nc.vector.tensor_scalar_mul(tvt[:], raw_q, 1.0 / D)
                nc.vector.tensor_mul(t2t[:], mur[0:1, 0:4], mur[0:1, 0:4])
                nc.vector.tensor_sub(t2t[:], tvt[:], t2t[:])
                nc.scalar.activation(t2t[:], t2t[:], AF.Ln, bias=epsb[0:1, :])
                nc.scalar.activation(mur[0:1, 4:8], t2t[:], AF.Exp, scale=-0.5)
                nc.tensor.matmul(misc4[0:SPC, MMU:MMU + 1],
                                 lhsT=mur[0:1, 0:4], rhs=onesr[0:1, 0:1],
                                 start=True, stop=True)
                nc.tensor.matmul(misc4[0:SPC, MMU + 1:MMU + 2],
                                 lhsT=mur[0:1, 4:8], rhs=onesr[0:1, 0:1],
                                 start=True, stop=True)
                murT = sm.tile([SPC, 2], F32, tag="murT", name="murT_t")
                nc.vector.tensor_copy(murT[:], misc4[0:SPC, MMU:MMU + 2])
                tout = sm.tile([SPC, PRED], F32, tag="tout", name="tout_t")
                nc.vector.tensor_scalar(tout[:], ocs4, murT[:, 0:1], None,
                                        op0=OP.mult)
                nc.vector.tensor_sub(tout[:], misc4[0:SPC, MHR:MHR + PRED],
                                     tout[:])
                outp = sm.tile([SPC, PRED], F32, tag="outp", name="outp_t")
                nc.scalar.activation(outp[:], tout[:], AF.Copy,
                                     scale=murT[:, 1:2])
                nc.sync.dma_start(Yout.ap(), outp[:])

            # ------- per-pair front sections -------
            def front_hi(pair):
                stats_mm(pair, [7])
                q0_mm(pair)
                q0_evict(pair)
                stats_mm(pair, [6, 5, 4])
                v0_mm(pair, 0, [4, 5, 6, 7])
                v0_mm(pair, 1, [4, 5, 6, 7])
                scores_mm(pair, 0, [4, 5, 6, 7])
                scores_mm(pair, 1, [4, 5, 6, 7])
                cA_mm(pair)
                if pair == 1:
                    v0_evict(1, 0, half=1)
                    v0_evict(1, 1, half=1)

            def front_lo(pair):
                stats_mm(pair, [3, 2, 1, 0])
                scores_mm(pair, 0, [0, 1, 2, 3])
                scores_mm(pair, 1, [0, 1, 2, 3])
                v0_mm(pair, 0, [0, 1, 2, 3])
                v0_mm(pair, 1, [0, 1, 2, 3])
                if pair == 0:
                    v0_evict(0, 0)
                    v0_evict(0, 1)
                else:
                    v0_evict(1, 0, half=0)
                    v0_evict(1, 1, half=0)

            # ================= emission schedule =================
            # Sample-sequential convs matching the X DMA arrival order; all
            # chain-dependent PE work slots where its inputs are already
            # ready so the in-order engine queues never head-of-line block.
            conv_m(0, 0, 3)
            evict_m(0, 0, 3)
            conv_m(0, 0, 2)
            evict_m(0, 0, 2)
            conv_m(0, 0, 1)
            evict_m(0, 0, 1)
            conv_m(0, 0, 0)
            evict_m(0, 0, 0)
            conv_m(0, 1, 3)
            evict_m(0, 1, 3)
            j7_stats_mm(0)
            conv_m(0, 1, 2)
            evict_m(0, 1, 2)
            conv_m(0, 1, 1)
            evict_m(0, 1, 1)
            conv_m(0, 1, 0)
            evict_m(0, 1, 0)
            front_hi(0)
            front_lo(0)
            conv_m(1, 0, 3)
            evict_m(1, 0, 3)
            conv_m(1, 0, 2)
            evict_m(1, 0, 2)
            conv_m(1, 1, 3)
            evict_m(1, 1, 3)
            j7_stats_mm(1)
            conv_m(1, 1, 2)
            evict_m(1, 1, 2)
            front_hi(1)
            j7_mid_a()
            conv_m(1, 0, 1)
            evict_m(1, 0, 1)
            conv_m(1, 0, 0)
            evict_m(1, 0, 0)
            j7_mid_b()
            j7_mid_c()
            conv_m(1, 1, 1)
            evict_m(1, 1, 1)
            conv_m(1, 1, 0)
            evict_m(1, 1, 0)
            front_lo(1)
            chain4()
            tail4()

    nc.compile()
    return nc


# cB = sum(arow) -- a host-side constant baked into the kernel IR.
CB_SUM_AROW = [0.0]

_NC_CACHE = {}


def _get_nc():
    if "nc" not in _NC_CACHE:
        _NC_CACHE["nc"] = build_nc()
    return _NC_CACHE["nc"]


def _prep_in_maps(inputs):
    f32 = np.float32
    X = np.ascontiguousarray(inputs["X"], f32)
    conv_w = np.asarray(inputs["conv_w"], f32)
    conv_b = np.asarray(inputs["conv_b"], f32)
    qw, kw, vw, ow = (np.asarray(inputs[k], f32) for k in ("qw", "kw", "vw", "ow"))
    expert_w = np.asarray(inputs["expert_w"], f32)
    expert_b = np.asarray(inputs["expert_b"], f32)
    router_w = np.asarray(inputs["router_w"], f32)
    moe_out_w = np.asarray(inputs["moe_out_w"], f32)
    out_w = np.asarray(inputs["out_w"], f32)

    nb = np.dtype(mybir.dt.np(BF16))
    n8 = np.dtype(mybir.dt.np(FP8E4))
    n5 = np.dtype(mybir.dt.np(FP8E5))
    nh = np.dtype(mybir.dt.np(FP16))

    # --- conv weights: fp8 hi (64x scale), 1x for the Xl stream, e5m2 resid
    Wc = conv_w.transpose(2, 1, 0).reshape(C * P, D)  # rows (p*64+c) -> k*128+r
    def chunked(w):
        return np.ascontiguousarray(
            w.reshape(NCH, 128, D).transpose(1, 0, 2).reshape(128, NCH * D))
    W8 = (64.0 * Wc).astype(n8)
    Wr8 = (64.0 * Wc - W8.astype(f32)).astype(n5)
    Wd8 = Wc.astype(n8)
    C8 = np.zeros((128, C8W), n8)
    C8[:, C8_W8:C8_W8 + 768] = chunked(W8.astype(f32)).astype(n8)
    C8[:, C8_WD:C8_WD + 768] = chunked(Wd8.astype(f32)).astype(n8)
    C5 = chunked(Wr8.astype(f32)).astype(n5)

    # --- CB (bf16) ---
    CB = np.zeros((128, CBW), f32)
    A_T = qw.T @ kw                    # lhsT for q0 = A @ h0_last
    CB[:, CB_AT:CB_AT + D] = np.concatenate([A_T, A_T], axis=0)
    vwT = vw.T
    CB[:, CB_VWT:CB_VWT + D] = np.concatenate([vwT, vwT], axis=0)
    CB[0:64, CB_SEL] = 1.0
    CB[64:128, CB_SEL + 1] = 1.0
    arow = (kw.T @ qw.sum(1)) / 8.0    # A @ 1, with the 1/sqrt(D) folded in
    CB[:, CB_AROW] = np.concatenate([arow, arow], axis=0)
    CB[:, CB_ONE] = 1.0
    # moe_out_w folded into the expert weights and biases
    Wme = np.einsum('ik,ekj->eij', moe_out_w, expert_w)
    # Wme[e, o2, d_in] = sum_o moe_out_w[o2, o] * expert_w[e, o, d_in]
    bme = expert_b @ moe_out_w.T       # (E, D)
    WexpE = np.concatenate(
        [Wme.transpose(2, 0, 1).reshape(D, E * D),
         bme.reshape(1, E * D)], axis=0)
    CB[0:D + 1, CB_WEXP:CB_WEXP + E * D] = WexpE
    CB_SUM_AROW[0] = float(arow.sum())

    # --- PB (fp16): 64*(pe + conv_b).T doubled rows ---
    PB = np.zeros((128, PBW), f32)
    pebT = 64.0 * (_pos_encoding_np(N, D) + conv_b[None, :]).T  # (64, N)
    PB[0:64, 0:N] = pebT
    PB[64:128, 0:N] = pebT

    # --- CF (f32) ---
    CF = np.zeros((128, CFW), f32)
    CF[0:D, CF_OWT:CF_OWT + D] = ow.T
    CF[0:D, CF_OUTWT:CF_OUTWT + PRED] = out_w.T
    CF[0:D, CF_RWT:CF_RWT + E] = router_w.T
    CF[0:D, CF_SV] = vw.sum(1)
    CF[126, CF_OH] = 1.0
    CF[0:4, CF_ID4:CF_ID4 + 4] = np.eye(4, dtype=f32)
    CF[0:8, CF_ID8:CF_ID8 + 8] = np.eye(8, dtype=f32)
    CF[0:1, CF_ONESR:CF_ONESR + 128] = 1.0
    CF[0:D, CF_ONE64] = 1.0
    ocs = out_w.T.sum(0)               # (96,)
    CF[0:SPC, CF_OCS:CF_OCS + PRED] = np.stack([ocs] * SPC, axis=0)
    CF[:, CF_LASTM] = 1.0
    CF[127, CF_LASTM] = 0.0

    # --- X: host-deinterleaved even/odd columns, fp8 hi + 64x residual ---
    Xr = X.reshape(B, C, LH, 2)
    common = dict(
        CB=np.ascontiguousarray(CB).astype(nb),
        C8=np.ascontiguousarray(C8),
        C5=np.ascontiguousarray(C5),
        PB=np.ascontiguousarray(PB).astype(nh),
        CF=np.ascontiguousarray(CF),
    )
    in_maps = []
    for c in range(NCORES):
        m = dict(common)
        xr = Xr[c * SPC:(c + 1) * SPC]
        xd = np.concatenate([xr[..., 0], xr[..., 1]], axis=1)  # (SPC, 128, LH)
        xd = np.ascontiguousarray(xd)
        xh8 = xd.astype(n8)
        m["Xh"] = xh8
        # last-token residual: Xd cols 6132:6144 per sample -> [128, 4*12]
        xl = 64.0 * (xd[:, :, LH - 12:LH] - xh8[:, :, LH - 12:LH].astype(f32))
        m["XlL"] = np.ascontiguousarray(
            xl.transpose(1, 0, 2).reshape(128, SPC * 12)).astype(n8)
        in_maps.append(m)
    return in_maps


def kernel(**inputs) -> np.ndarray:
    in_maps = _prep_in_maps(inputs)
    nc = _get_nc()
    res = run_bass_kernel_spmd(nc, in_maps, core_ids=list(range(NCORES)))
    out = np.concatenate([res.results[c]["Yout"] for c in range(NCORES)], axis=0)
    return out.astype(np.float32)


# revision 72
# speedup vs baseline: 1.4043x; 1.0518x over previous
"""Trainium2 Bass kernel for nn_Decoder_22703197127089 (moe_routing).

Only the last token survives to the output (h[:, -1, :] is taken after the
MoE block), so per sample we need: conv patch-embed for all 1023 tokens
(feeds K/V), folded-LN attention row for the last token, MoE + LN2 + head
for one token.

v3 design notes (on top of the v2 folded-LN / deinterleaved-X baseline):
  - The conv runs in fp8e4 DoubleRow mode (0.5 cyc/row): X is shipped as
    fp8 hi (Xh) plus a 64x-scaled fp8 residual (Xl); conv weights ride as
    fp8e4(64*W) (pairs Xh), fp8e4(W) (pairs Xl), and an e5m2 second-order
    residual fp8e5(64W - fp8(64W)) (pairs Xh).  psum accumulates 64*h0;
    the 64x scale cancels through the folded-LN score algebra, so only
    pebt (x64) and the LN eps (x64^2) change.
  - The whole attention/MoE back-end is batched 4-wide: one shared stats
    pipeline, one z-reduction, one oc/proj, one MoE tail for all 4
    samples, cutting the exposed end-of-kernel serial chain in half.
  - moe_out_w is folded into the expert weights host-side.
  - z-reduction uses a single colsum matmul + grouped tensor_reduce;
    scalar_tensor_tensor fuses the mean/var and c' arithmetic.

Sharding: data-parallel over batch B=32 across 8 cores (4 samples/core),
host gathers (4, 96) per-core outputs.
"""

import numpy as np

import concourse.bass as bass
import concourse.mybir as mybir
import concourse.tile as tile
from concourse import bacc
from concourse.bass_utils import run_bass_kernel_spmd

F32 = mybir.dt.float32
BF16 = mybir.dt.bfloat16
FP16 = mybir.dt.float16
FP8E4 = mybir.dt.float8e4
FP8E5 = mybir.dt.float8e5
DR = mybir.MatmulPerfMode.DoubleRow
AF = mybir.ActivationFunctionType
OP = mybir.AluOpType
AX = mybir.AxisListType

B, C, L = 32, 64, 12288
D = 64
E = 8
TOPK = 4
P, S = 24, 12
PRED = 96
N = (L - P) // S + 1  # 1023
NCORES = 8
SPC = B // NCORES     # 4 samples per core
NPAIR = SPC // 2      # 2
NCH = (C * P) // 128  # 12 contraction chunks of K=128
NDR = NCH // 2        # 6 DoubleRow chunk-pairs (K=256 each)
NT = 1024             # padded token dim (col 1023 zeroed)
NJ = 8                # 128-token chunks
EPS = 1e-5
EPS64 = EPS * 64.0 * 64.0   # LN eps at the 64x h0 scale
LH = 6144             # deinterleaved X columns
XSPLIT = 3072         # X half-DMA split: m2/m3 read only [3072:6144]; m0/m1
                      # read [0:3078] (m1's last patch peeks 6 cols into h1,
                      # which always lands first)
MCS = (256, 256, 256, 255)  # conv m-chunk sizes (patches); DoubleRow rhs
                            # free size 2*nn must stay <= 512

# ---- CB (bf16) column map ----
CB_AT = 0             # A^T = qw.T @ kw, doubled rows        (64)
CB_VWT = 64           # vw.T doubled                          (64)
CB_SEL = 128          # selab                                 (2)
CB_AROW = 130         # arow doubled                          (1)
CB_ONE = 131          # ones column                           (1)
CB_WEXP = 136         # moe_out-folded experts (rows 0:65)    (512)
CBW = 648

# ---- C8 (fp8e4) column map: conv weights, 64x and 1x scale ----
C8_W8 = 0             # fp8e4(64*W) chunks                    (768)
C8_WD = 768           # fp8e4(W) chunks (pairs the Xl stream) (768)
C8W = 1536

# C5 (fp8e5): Wr8 = fp8e5(64*W - W8) chunks                   (768)
C5W = 768

# PB (fp16): 64*(pe + conv_b).T doubled rows, col 1023 zero   (1024)
PBW = 1024

# ---- CF (f32) column map ----
CF_OWT = 0            # ow.T rows 0:64                        (64)
CF_OUTWT = 64         # out_w.T rows 0:64                     (96)
CF_RWT = 160          # router_w.T rows 0:64                  (8)
CF_SV = 168           # vw.sum(1) rows 0:64                   (1)
CF_OH = 169           # onehot at partition 126               (1)
CF_ID4 = 170          # eye(4) rows 0:4                       (4)
CF_ID8 = 174          # eye(8) rows 0:8                       (8)
CF_ONESR = 182        # ones row 0                            (128)
CF_ONE64 = 310        # ones rows 0:64                        (1)
CF_OCS = 311          # out_w.T colsums rows 0:4              (96)
CF_LASTM = 407        # ones, 0 at partition 127              (1)
CFW = 408

# ---- stp4 (psum, shared) column map: 4-wide per-token stats ----
# sums 0:32 (col 4j+s), sumsq 32:64, sA 64:96, sB 96:128

# ---- misc4 (psum, shared) column map ----
MQ0 = 0               # q0 per sample                         (4)
MJ7R = 4              # j7 row [1,8]                          (8)
MB1 = 12              # bcast1: muL 12:16, rL 16:20           (8)
MB2 = 20              # bcast2: c'                            (4)
MCA = 24              # cA [1,4]                              (4)
MZR = 28              # zrow [1,64]                           (64)
MBZ = 92              # bcastz: rz 92:96, gz 96:100           (8)
MATT = 100            # attraw [64,4]                         (4)
MHA = 104             # proj ha [64,4]                        (4)
MLG = 108             # logits [4,8]                          (8)
MMT = 120             # moeT [64,4]                           (4)
MSM = 124             # LN2 sums [1,8]                        (8)
MMU = 132             # mur bcast [4,2]                       (2)
MHR = 160             # head raw [4,96]                       (96)


def _pos_encoding_np(n, d):
    pos = np.arange(n, dtype=np.float32)[:, None]
    div = np.exp(np.arange(0, d, 2, dtype=np.float32)
                 * (np.float32(-np.log(np.float32(10000.0))) / np.float32(d)))
    pe = np.zeros((n, d), np.float32)
    pe[:, 0::2] = np.sin(pos * div)
    pe[:, 1::2] = np.cos(pos * div)
    return pe


def build_nc():
    nc = bacc.Bacc("TRN2", target_bir_lowering=False, debug=False,
                   num_devices=NCORES)

    Xht = nc.dram_tensor("Xh", [SPC, 128, LH], FP8E4, kind="ExternalInput")
    XlLt = nc.dram_tensor("XlL", [128, SPC * 12], FP8E4, kind="ExternalInput")
    CBt = nc.dram_tensor("CB", [128, CBW], BF16, kind="ExternalInput")
    C8t = nc.dram_tensor("C8", [128, C8W], FP8E4, kind="ExternalInput")
    C5t = nc.dram_tensor("C5", [128, C5W], FP8E5, kind="ExternalInput")
    PBt = nc.dram_tensor("PB", [128, PBW], FP16, kind="ExternalInput")
    CFt = nc.dram_tensor("CF", [128, CFW], F32, kind="ExternalInput")
    Yout = nc.dram_tensor("Yout", [SPC, PRED], F32, kind="ExternalOutput")

    with tile.TileContext(nc) as tc:
        with (
            tc.tile_pool(name="const", bufs=1) as pc,
            tc.tile_pool(name="xp", bufs=4) as xp,
            tc.tile_pool(name="hp", bufs=4) as hp,
            tc.tile_pool(name="sm", bufs=4) as sm,
            tc.tile_pool(name="vp", bufs=4) as vp,
            tc.tile_pool(name="ps", bufs=1, space="PSUM") as ps,
        ):
            # ---------------- constants / warmup ----------------
            cb = pc.tile([128, CBW], BF16, tag="cb")
            c8 = pc.tile([128, C8W], FP8E4, tag="c8")
            c5 = pc.tile([128, C5W], FP8E5, tag="c5")
            pb = pc.tile([128, PBW], FP16, tag="pb")
            cf = pc.tile([128, CFW], F32, tag="cf")
            junk = pc.tile([128, 512], BF16, tag="junk")
            epsb = pc.tile([128, 1], F32, tag="epsb")
            eps64b = pc.tile([128, 1], F32, tag="eps64b")

            xht = []
            for s in range(SPC):
                xht.append(xp.tile([128, LH], FP8E4, tag="xh", name="xh%d" % s))
            xll = xp.tile([128, SPC * 12], FP8E4, tag="xll", name="xll")

            def xdma(s, half):
                t = xht[s]
                src = Xht.ap()[s]
                if half == 0:
                    nc.sync.dma_start(t[:, 0:XSPLIT], src[:, 0:XSPLIT])
                else:
                    nc.sync.dma_start(t[:, XSPLIT:LH], src[:, XSPLIT:LH])

            # DMA order: conv weights first, then per-sample halves, upper
            # (h1) halves before lower (m2/m3 hold the last token + j7
            # stats); small consts ride between X transfers; the expert
            # weights (only needed by the tail) go last.
            xdma(0, 1)
            nc.sync.dma_start(c8[:], C8t.ap())
            nc.sync.dma_start(c5[:], C5t.ap())
            nc.sync.dma_start(xll[:], XlLt.ap())
            xdma(0, 0)
            nc.sync.dma_start(pb[:], PBt.ap())
            nc.sync.dma_start(cb[:, 0:CB_WEXP], CBt.ap()[:, 0:CB_WEXP])
            xdma(1, 1)
            xdma(1, 0)
            nc.sync.dma_start(cf[:], CFt.ap())
            xdma(2, 1)
            xdma(3, 1)
            xdma(2, 0)
            xdma(3, 0)
            nc.sync.dma_start(cb[:, CB_WEXP:CBW], CBt.ap()[:, CB_WEXP:CBW])

            nc.vector.memset(junk[:], 0.25)
            nc.vector.memset(epsb[:], EPS)
            nc.vector.memset(eps64b[:], EPS64)

            # Pre-load the one act-function set covering Square/Exp/Ln/Copy
            from concourse.hw_specs import get_activation_tables
            _set_id = list(get_activation_tables(nc.m.arch)).index(
                "natural_log_exp_and_others")
            nc.scalar.add_instruction(mybir.InstLoadActFuncSet(
                name=nc.get_next_instruction_name(), ins=[], outs=[],
                act_func_set_id=_set_id))

            w8v = c8[:, C8_W8:C8_W8 + 768].rearrange("p (k d) -> p k d", k=NCH)
            wdv = c8[:, C8_WD:C8_WD + 768].rearrange("p (k d) -> p k d", k=NCH)
            wrv = c5[:, 0:768].rearrange("p (k d) -> p k d", k=NCH)
            at2 = cb[:, CB_AT:CB_AT + 64]
            vwt2 = cb[:, CB_VWT:CB_VWT + 64]
            selab = cb[:, CB_SEL:CB_SEL + 2]
            arow2 = cb[:, CB_AROW:CB_AROW + 1]
            onesb = cb[:, CB_ONE:CB_ONE + 1]
            lastmf = cf[:, CF_LASTM:CF_LASTM + 1]
            wexpb = cb[0:D + 1, CB_WEXP:CB_WEXP + E * D]
            owt = cf[0:D, CF_OWT:CF_OWT + D]
            outwt = cf[0:D, CF_OUTWT:CF_OUTWT + PRED]
            rwt = cf[0:D, CF_RWT:CF_RWT + E]
            svcol = cf[0:D, CF_SV:CF_SV + 1]
            onehotf = cf[:, CF_OH:CF_OH + 1]
            id4 = cf[0:4, CF_ID4:CF_ID4 + 4]
            id8 = cf[0:8, CF_ID8:CF_ID8 + 8]
            onesr = cf[0:1, CF_ONESR:CF_ONESR + 128]
            ones64 = cf[0:D, CF_ONE64:CF_ONE64 + 1]
            ocs4 = cf[0:SPC, CF_OCS:CF_OCS + PRED]

            # PE warmup: keep the tensor engine busy during initial DMA so
            # real matmuls dispatch at full p-state.
            jp = ps.tile([128, 512], F32, tag="v0p", name="junkp", bufs=2)
            jp2 = ps.tile([128, 512], F32, tag="v0p", name="junkp2", bufs=2)
            for i in range(7):
                t = jp if i % 2 == 0 else jp2
                nc.tensor.matmul(t[:, 0:512], lhsT=junk[:, 0:128],
                                 rhs=junk[:, 0:512], start=True, stop=True)

            # Shared attention-output tiles (row 64 = 1.0 feeds expert bias)
            haf = pc.tile([D + 1, SPC], F32, tag="haf")
            hab = pc.tile([D + 1, SPC], BF16, tag="hab")
            nc.vector.memset(haf[D:D + 1, :], 1.0)
            nc.vector.memset(hab[D:D + 1, :], 1.0)

            # Shared 4-wide attention state
            stp4 = ps.tile([128, 128], F32, tag="stp", name="stp4")
            misc4 = ps.tile([128, 256], F32, tag="misc", name="misc4")
            eg4 = sm.tile([128, 96], BF16, tag="eg", name="eg4")
            PR = [dict() for _ in range(NPAIR)]
            SH = {}

            # ---------------- emission helpers ----------------
            def conv_m(pair, s01, mi):
                """12 DoubleRow accumulation matmuls for sample (2*pair+s01),
                m-chunk mi: 6 chunk-pairs x {Xh*W8, Xh*Wr8}; psum holds
                64*h0.  m3 adds 6 narrow Xl*Wd8 DRs correcting the last
                token (col 510).  m-chunks {0,1} share cpsL, {2,3} cpsH."""
                st = PR[pair]
                s = 2 * pair + s01
                n0 = 256 * mi
                nn = MCS[mi]
                # DoubleRow dst partitions must start at 0 (s3d3 ISA rule),
                # so every sample gets its own base-0 psum tile.
                key = "cps%d_%d" % (mi, s01)
                if key not in st:
                    st[key] = ps.tile([64, 256], F32, tag="cps",
                                      name="cps_p%d_%s" % (pair, key), bufs=3)
                cps = st[key]
                # [128, 6(r), 1024(n)]: dim1 = the DoubleRow k-tile pair
                xvh = xht[s][:].rearrange("p (n s) -> p s n", s=6)
                mms = ([(xvh, w8v, j) for j in range(NDR)]
                       + [(xvh, wrv, j) for j in range(NDR)])
                last = mi != 3
                for i, (xv, wv, j) in enumerate(mms):
                    q, r = divmod(2 * j, 6)
                    nc.tensor.matmul(
                        cps[0:64, 0:nn],
                        lhsT=wv[:, 2 * j:2 * j + 2, :],
                        rhs=xv[:, r:r + 2, n0 + q:n0 + q + nn],
                        start=(i == 0), stop=(last and i == len(mms) - 1),
                        perf_mode=DR)
                if mi == 3:
                    # last-token (1022 -> psum col 254) fp8-residual fixup:
                    # 12 plain single-column fp8 matmuls, one per K-chunk
                    xlv = xll[:].rearrange("p (s q r) -> p s q r",
                                           s=SPC, q=2, r=6)
                    for k in range(NCH):
                        q, r = divmod(k, 6)
                        nc.tensor.matmul(
                            cps[0:64, 254:255],
                            lhsT=wdv[:, k, :],
                            rhs=xlv[:, s, q, r:r + 1],
                            start=False, stop=(k == NCH - 1),
                            skip_group_check=True)

            def evict_m(pair, s01, mi):
                """psum + pebt64 -> h0b (bf16, 64x scale) on DVE; Square ->
                sq on Act.  Per-(sample, m-chunk) granularity: base-0 tiles
                keep DVE/Act lanes aligned with the base-0 conv psum."""
                st = PR[pair]
                hk, qk = "h0b%d" % s01, "sq%d" % s01
                if hk not in st:
                    st[hk] = hp.tile([64, NT], BF16, tag="h0b",
                                     name="h0b_p%d_s%d" % (pair, s01))
                    st[qk] = hp.tile([64, NT], BF16, tag="sq",
                                     name="sq_p%d_s%d" % (pair, s01))
                    nc.vector.memset(st[hk][:, N:NT], 0.0)
                    nc.vector.memset(st[qk][:, N:NT], 0.0)
                n0 = 256 * mi
                nn = MCS[mi]
                cps = st["cps%d_%d" % (mi, s01)]
                nc.vector.tensor_add(st[hk][:, n0:n0 + nn],
                                     cps[0:64, 0:nn], pb[0:64, n0:n0 + nn])
                nc.scalar.activation(st[qk][:, n0:n0 + nn],
                                     st[hk][:, n0:n0 + nn], AF.Square)

            def stats_mm(pair, js):
                """Per-token sums / sum-of-squares into stp4 col 4j+s."""
                st = PR[pair]
                for j in js:
                    for s01 in range(2):
                        c = 4 * j + 2 * pair + s01
                        h0b, sq = st["h0b%d" % s01], st["sq%d" % s01]
                        nc.tensor.matmul(stp4[:, c:c + 1],
                                         lhsT=h0b[:, 128 * j:128 * j + 128],
                                         rhs=onesb[0:64, :],
                                         start=True, stop=True)
                        nc.tensor.matmul(stp4[:, 32 + c:32 + c + 1],
                                         lhsT=sq[:, 128 * j:128 * j + 128],
                                         rhs=onesb[0:64, :],
                                         start=True, stop=True)

            def q0_mm(pair):
                for s01 in range(2):
                    s = 2 * pair + s01
                    nc.tensor.matmul(misc4[0:64, MQ0 + s:MQ0 + s + 1],
                                     lhsT=at2[0:64, :],
                                     rhs=PR[pair]["h0b%d" % s01][:, N - 1:N],
                                     start=True, stop=True)

            def q0_evict(pair):
                # 1/sqrt(D)=1/8 score scale folded here (and into host arow)
                st = PR[pair]
                for s01 in range(2):
                    s = 2 * pair + s01
                    t = sm.tile([64, 1], BF16, tag="q0sb",
                                name="q0sb_s%d" % s)
                    st["q0sb%d" % s01] = t
                    nc.vector.tensor_scalar_mul(
                        t[:], misc4[0:64, MQ0 + s:MQ0 + s + 1], 0.125)

            def cA_mm(pair):
                # cA = sum(q0) per sample -> misc4[0:1, MCA+s]
                for s01 in range(2):
                    s = 2 * pair + s01
                    nc.tensor.matmul(misc4[0:1, MCA + s:MCA + s + 1],
                                     lhsT=PR[pair]["q0sb%d" % s01][:],
                                     rhs=onesb[0:64, :],
                                     start=True, stop=True)

            def v0_mm(pair, s01, js):
                st = PR[pair]
                key = "v0p%d" % s01
                if key not in st:
                    st[key] = ps.tile([128, 512], F32, tag="v0p",
                                      name="v0_p%d_s%d" % (pair, s01), bufs=2)
                v0p = st[key]
                h0b = st["h0b%d" % s01]
                for j in js:
                    nc.tensor.matmul(v0p[:, 64 * j:64 * j + 64],
                                     lhsT=h0b[:, 128 * j:128 * j + 128],
                                     rhs=vwt2[0:64, :],
                                     start=True, stop=True)

            def v0_evict(pair, s01, half=None):
                """psum->SBUF staging for the attention values.  Pair-0 rides
                the idle Pool engine; pair-1 (latency-critical) is copied in
                hi/lo halves on DVE/Act in their idle windows."""
                st = PR[pair]
                key = "v0sb%d" % s01
                if key not in st:
                    st[key] = vp.tile([128, 512], BF16, tag="v0sb",
                                      name="v0sb_t")
                if half is None:
                    # GPSIMD cannot touch PSUM; split pair-0's copies over
                    # the DVE/Act queues (off the critical window).
                    if s01 == 0:
                        nc.scalar.copy(st[key][:], st["v0p%d" % s01][:])
                    else:
                        nc.vector.tensor_copy(st[key][:], st["v0p%d" % s01][:])
                    return
                c0, c1 = (256, 512) if half else (0, 256)
                if s01 == 0:
                    nc.vector.tensor_copy(st[key][:, c0:c1],
                                          st["v0p%d" % s01][:, c0:c1])
                else:
                    nc.scalar.copy(st[key][:, c0:c1],
                                   st["v0p%d" % s01][:, c0:c1])

            def scores_mm(pair, which, js):
                """which=0: sA = h0b^T q0 (stp4 64+); 1: sB = h0b^T arow (96+)."""
                st = PR[pair]
                base = 64 if which == 0 else 96
                for j in js:
                    for s01 in range(2):
                        c = base + 4 * j + 2 * pair + s01
                        h0b = st["h0b%d" % s01]
                        rhs = (st["q0sb%d" % s01][:] if which == 0
                               else arow2[0:64, :])
                        nc.tensor.matmul(
                            stp4[:, c:c + 1],
                            lhsT=h0b[:, 128 * j:128 * j + 128],
                            rhs=rhs, start=True, stop=True)

            # ------- 4-wide mid: last-token stats for all samples -------
            # Split a/b/c so the PE pieces interleave with pair-1 conv groups
            # without head-of-line blocking them.
            def j7_stats_mm(pair):
                """Row-form sum / sumsq of h0b col 1022 (K=64, M=1 matmuls
                with the h0b column itself as rhs for the square)."""
                for s01 in range(2):
                    s = 2 * pair + s01
                    col = PR[pair]["h0b%d" % s01][:, N - 1:N]
                    nc.tensor.matmul(misc4[0:1, MJ7R + s:MJ7R + s + 1],
                                     lhsT=col, rhs=onesb[0:64, :],
                                     start=True, stop=True)
                    nc.tensor.matmul(misc4[0:1, MJ7R + 4 + s:MJ7R + 5 + s],
                                     lhsT=col, rhs=col,
                                     start=True, stop=True)

            def j7_mid_a():
                """j7 raw row stats -> mb4 = [muL x4 | rL x4]; c'."""
                mb4 = sm.tile([1, 8], F32, tag="mb", name="mb4_t")
                SH["mb4"] = mb4
                t2j = sm.tile([1, 4], F32, tag="t2j", name="t2j_t")
                raw_s = misc4[0:1, MJ7R:MJ7R + 4]
                raw_q = misc4[0:1, MJ7R + 4:MJ7R + 8]
                nc.vector.tensor_scalar_mul(mb4[0:1, 0:4], raw_s, 1.0 / D)
                nc.vector.tensor_mul(t2j[:], mb4[0:1, 0:4], mb4[0:1, 0:4])
                tvj = sm.tile([1, 4], F32, tag="tvj", name="tvj_t")
                nc.vector.tensor_scalar_mul(tvj[:], raw_q, 1.0 / D)
                nc.vector.tensor_sub(t2j[:], tvj[:], t2j[:])
                nc.scalar.activation(t2j[:], t2j[:], AF.Ln, bias=eps64b[0:1, :])
                nc.scalar.activation(mb4[0:1, 4:8], t2j[:], AF.Exp, scale=-0.5)
                # c' = cA - muL*cB (the bcast rides in j7_mid_c)
                cpr4 = sm.tile([1, 4], F32, tag="cpr", name="cpr4_t")
                SH["cpr4"] = cpr4
                nc.vector.tensor_scalar_mul(cpr4[:], SH["mb4"][0:1, 0:4],
                                            -CB_SUM_AROW[0])
                nc.vector.tensor_add(cpr4[:], cpr4[:], misc4[0:1, MCA:MCA + 4])

            def j7_mid_b():
                # broadcast muL/rL to 128 partitions; SBUF copy since the
                # assemble ops already read stp4 from PSUM (one-PSUM rule)
                nc.tensor.matmul(misc4[:, MB1:MB1 + 8], lhsT=onesr,
                                 rhs=SH["mb4"][:], start=True, stop=True)
                bcsb4 = sm.tile([128, 8], F32, tag="bcsb", name="bcsb4_t")
                SH["bcsb4"] = bcsb4
                nc.vector.tensor_copy(bcsb4[:], misc4[:, MB1:MB1 + 8])

            def j7_mid_c():
                nc.tensor.matmul(misc4[:, MB2:MB2 + 4], lhsT=onesr,
                                 rhs=SH["cpr4"][:], start=True, stop=True)

            # ------- 4-wide chain: stats -> exps -> z -> att -> proj -------
            def chain4():
                mean4 = sm.tile([128, 32], F32, tag="mean", name="mean4_t")
                rstd4 = sm.tile([128, 32], F32, tag="rstd", name="rstd4_t")
                t2a = sm.tile([128, 32], F32, tag="tmpa", name="t2a_t")
                nc.vector.tensor_scalar_mul(mean4[:], stp4[:, 0:32], 1.0 / D)
                tva = sm.tile([128, 32], F32, tag="tva", name="tva_t")
                nc.vector.tensor_scalar_mul(tva[:], stp4[:, 32:64], 1.0 / D)
                nc.vector.tensor_mul(t2a[:], mean4[:], mean4[:])
                nc.vector.tensor_sub(t2a[:], tva[:], t2a[:])
                nc.scalar.activation(t2a[:], t2a[:], AF.Ln, bias=eps64b[:])
                nc.scalar.activation(rstd4[:], t2a[:], AF.Exp, scale=-0.5)

                # sc = (sA - muL*sB - mu*c') * rstd * rL ; exps = exp(sc).
                # t3 rides on DVE during the Ln/Exp window; everything stays
                # on DVE to avoid cross-engine sem hops on the serial chain.
                bc = SH["bcsb4"]
                v4 = lambda ap: ap.rearrange("p (j s) -> p j s", s=4)
                t1 = sm.tile([128, 32], F32, tag="t1", name="t1_t")
                t3 = sm.tile([128, 32], F32, tag="t3", name="t3_t")
                nc.vector.tensor_tensor(
                    v4(t3[:]), v4(mean4[:]),
                    misc4[:, MB2:MB2 + 4][:, None].to_broadcast([128, NJ, 4]),
                    op=OP.mult)
                nc.vector.tensor_tensor(
                    v4(t1[:]), v4(stp4[:, 96:128]),
                    bc[:, 0:4][:, None].to_broadcast([128, NJ, 4]), op=OP.mult)
                nc.vector.tensor_sub(t1[:], stp4[:, 64:96], t1[:])
                nc.vector.tensor_sub(t1[:], t1[:], t3[:])
                r8 = sm.tile([128, 32], F32, tag="r8", name="r8_t")
                nc.vector.tensor_tensor(
                    v4(r8[:]), v4(rstd4[:]),
                    bc[:, 4:8][:, None].to_broadcast([128, NJ, 4]), op=OP.mult)
                nc.vector.tensor_mul(t1[:], t1[:], r8[:])
                # eg4 = [exps 0:32 | gt 32:64 | et 64:96].  The padded token
                # 1023 yields sc=0 -> exp=1 exactly; its +1 in Z is removed
                # below and it contributes nothing to att (v0 col is 0) or
                # gs (mean is 0), so no mask op is needed.
                nc.scalar.activation(eg4[:, 0:32], t1[:], AF.Exp)
                nc.vector.tensor_mul(eg4[:, 64:96], eg4[:, 0:32], rstd4[:])
                nc.vector.tensor_mul(eg4[:, 32:64], eg4[:, 64:96], mean4[:])

                # z-reduction: colsums -> grouped j-reduce -> rz, gz
                nc.tensor.matmul(
                    misc4[0:1, MZR:MZR + 64], lhsT=onesb,
                    rhs=eg4[:, 0:64].rearrange("p (g j s) -> p g s j",
                                               g=2, j=NJ, s=4),
                    start=True, stop=True)
                zg = sm.tile([1, 8], F32, tag="zg", name="zg_t")
                nc.vector.tensor_reduce(
                    zg[:],
                    misc4[0:1, MZR:MZR + 64].rearrange("p (a j) -> p a j", a=8),
                    AX.X, OP.add)
                # Z carries the pad token's exp(0)=1: subtract it here
                nc.vector.tensor_scalar_add(zg[0:1, 0:4], zg[0:1, 0:4], -1.0)
                rg4 = sm.tile([1, 8], F32, tag="rg", name="rg4_t")
                nc.vector.reciprocal(rg4[0:1, 0:4], zg[0:1, 0:4])
                nc.vector.tensor_mul(rg4[0:1, 4:8], zg[0:1, 4:8],
                                     rg4[0:1, 0:4])
                nc.tensor.matmul(misc4[0:64, MBZ:MBZ + 8],
                                 lhsT=onesr[0:1, 0:64], rhs=rg4[:],
                                 start=True, stop=True)
                bz = sm.tile([64, 8], F32, tag="bz", name="bz4_t")
                nc.vector.tensor_copy(bz[:], misc4[0:64, MBZ:MBZ + 8])
                bz = bz[:]

                # attraw per sample
                for s in range(SPC):
                    pair, s01 = divmod(s, 2)
                    v0sb = PR[pair]["v0sb%d" % s01]
                    for j in range(NJ):
                        nc.tensor.matmul(
                            misc4[0:64, MATT + s:MATT + s + 1],
                            lhsT=v0sb[:, 64 * j:64 * j + 64],
                            rhs=eg4[:, 64 + 4 * j + s:65 + 4 * j + s],
                            start=(j == 0), stop=(j == NJ - 1))

                # oc = attraw*rz - sv*gz ; project through ow.T
                svz = sm.tile([64, 4], F32, tag="svz", name="svz_t")
                nc.vector.tensor_tensor(svz[:], svcol.to_broadcast([D, SPC]),
                                        bz[:, 4:8], op=OP.mult)
                tt4 = sm.tile([64, 4], F32, tag="tt4", name="tt4_t")
                nc.vector.tensor_tensor(tt4[:], misc4[0:64, MATT:MATT + 4],
                                        bz[:, 0:4], op=OP.mult)
                oc2 = sm.tile([64, 4], F32, tag="oc", name="oc2_t")
                nc.vector.tensor_sub(oc2[:], tt4[:], svz[:])
                nc.tensor.matmul(misc4[0:64, MHA:MHA + 4], lhsT=owt,
                                 rhs=oc2[:], start=True, stop=True)
                nc.vector.tensor_copy(haf[0:D, :], misc4[0:64, MHA:MHA + 4])
                nc.scalar.copy(hab[0:D, :], misc4[0:64, MHA:MHA + 4])

            # ------- 4-wide tail: router/topk -> experts -> LN2 -> head -----
            def tail4():
                nc.tensor.matmul(misc4[0:SPC, MLG:MLG + 8], lhsT=haf[0:D, :],
                                 rhs=rwt, start=True, stop=True)
                lg = misc4[0:SPC, MLG:MLG + 8]
                m8 = sm.tile([SPC, 8], F32, tag="m8", name="m8_t")
                nc.vector.max(m8[:], lg)
                msk = sm.tile([SPC, 8], F32, tag="msk", name="msk_t")
                nc.vector.tensor_scalar(msk[:], lg, m8[:, TOPK - 1:TOPK],
                                        None, op0=OP.is_ge)
                el = sm.tile([SPC, 8], F32, tag="el", name="el_t")
                nc.scalar.activation(el[:], lg, AF.Exp)
                w4 = sm.tile([SPC, 8], F32, tag="w4", name="w4_t")
                nc.vector.tensor_mul(w4[:], el[:], msk[:])
                eop = ps.tile([128, 512], F32, tag="eop", name="eop_t")
                nc.tensor.matmul(eop[0:SPC, :], lhsT=hab[:], rhs=wexpb,
                                 start=True, stop=True)
                prod = sm.tile([SPC, E * D], F32, tag="prod", name="prod_t")
                moe4 = sm.tile([SPC, D], F32, tag="moe4", name="moe4_t")
                nc.vector.tensor_tensor(
                    prod[:].rearrange("p (e d) -> p e d", e=E),
                    eop[0:SPC, :].rearrange("p (e d) -> p e d", e=E),
                    w4[:].to_broadcast([SPC, E, D]), op=OP.mult)
                nc.vector.tensor_reduce(
                    moe4[:], prod[:].rearrange("p (e d) -> p d e", e=E),
                    AX.X, OP.add)
                # h2 (moe_out folded into experts) -> LN2 -> head; sumsq via
                # K=64 M=1 matmuls using the hm column as its own rhs.
                nc.tensor.transpose(misc4[0:D, MMT:MMT + 4], moe4[:],
                                    id4[0:SPC, 0:SPC])
                hm8 = sm.tile([D, 2 * SPC], F32, tag="hm", name="hm8_t")
                nc.vector.tensor_copy(hm8[:, 0:4], misc4[0:D, MMT:MMT + 4])
                nc.tensor.matmul(misc4[0:1, MSM:MSM + 4], lhsT=ones64,
                                 rhs=hm8[:, 0:4], start=True, stop=True)
                for s in range(SPC):
                    nc.tensor.matmul(misc4[0:1, MSM + 4 + s:MSM + 5 + s],
                                     lhsT=hm8[0:D, s:s + 1],
                                     rhs=hm8[0:D, s:s + 1],
                                     start=True, stop=True)
                nc.tensor.matmul(misc4[0:SPC, MHR:MHR + PRED],
                                 lhsT=hm8[:, 0:4], rhs=outwt,
                                 start=True, stop=True)
                mur = sm.tile([1, 8], F32, tag="mur", name="mur_t")
                raw_s = misc4[0:1, MSM:MSM + 4]
                raw_q = misc4[0:1, MSM + 4:MSM + 8]
                nc.vector.tensor_scalar_mul(mur[0:1, 0:4], raw_s, 1.0 / D)
                t2t = sm.tile([1, 4], F32, tag="t2t", name="t2t_t")
                tvt = sm.tile([1, 4], F32, tag="tvt", name="tvt_t")
                nc.vector.tensor_scalar_mul(tvt[:], raw_q, 1.0 / D)
                nc.vector.tensor_mul(t2t[:], mur[0:1, 0:4], mur[0:1, 0:4])
                nc.vector.tensor_sub(t2t[:], tvt[:], t2t[:])
                nc.scalar.activation(t2t[:], t2t[:], AF.Ln, bias=epsb[0:1, :])
                nc.scalar.activation(mur[0:1, 4:8], t2t[:], AF.Exp, scale=-0.5)
                nc.tensor.matmul(misc4[0:SPC, MMU:MMU + 1],
                                 lhsT=mur[0:1, 0:4], rhs=onesr[0:1, 0:1],
                                 start=True, stop=True)
                nc.tensor.matmul(misc4[0:SPC, MMU + 1:MMU + 2],
                                 lhsT=mur[0:1, 4:8], rhs=onesr[0:1, 0:1],
                                 start=True, stop=True)
                murT = sm.tile([SPC, 2], F32, tag="murT", name="murT_t")
                nc.vector.tensor_copy(murT[:], misc4[0:SPC, MMU:MMU + 2])
                tout = sm.tile([SPC, PRED], F32, tag="tout", name="tout_t")
                nc.vector.tensor_scalar(tout[:], ocs4, murT[:, 0:1], None,
                                        op0=OP.mult)
                nc.vector.tensor_sub(tout[:], misc4[0:SPC, MHR:MHR + PRED],
                                     tout[:])
                outp = sm.tile([SPC, PRED], F32, tag="outp", name="outp_t")
                nc.scalar.activation(outp[:], tout[:], AF.Copy,
                                     scale=murT[:, 1:2])
                nc.sync.dma_start(Yout.ap(), outp[:])

            # ------- per-pair front sections -------
            def front_hi(pair):
                stats_mm(pair, [7])
                q0_mm(pair)
                q0_evict(pair)
                stats_mm(pair, [6, 5, 4])
                v0_mm(pair, 0, [4, 5, 6, 7])
                v0_mm(pair, 1, [4, 5, 6, 7])
                scores_mm(pair, 0, [4, 5, 6, 7])
                scores_mm(pair, 1, [4, 5, 6, 7])
                cA_mm(pair)
                if pair == 1:
                    v0_evict(1, 0, half=1)
                    v0_evict(1, 1, half=1)

            def front_lo(pair):
                stats_mm(pair, [3, 2, 1, 0])
                scores_mm(pair, 0, [0, 1, 2, 3])
                scores_mm(pair, 1, [0, 1, 2, 3])
                v0_mm(pair, 0, [0, 1, 2, 3])
                v0_mm(pair, 1, [0, 1, 2, 3])
                if pair == 0:
                    v0_evict(0, 0)
                    v0_evict(0, 1)
                else:
                    v0_evict(1, 0, half=0)
                    v0_evict(1, 1, half=0)

            # ================= emission schedule =================
            # Sample-sequential convs matching the X DMA arrival order; all
            # chain-dependent PE work slots where its inputs are already
            # ready so the in-order engine queues never head-of-line block.
            conv_m(0, 0, 3)
            evict_m(0, 0, 3)
            conv_m(0, 0, 2)
            evict_m(0, 0, 2)
            conv_m(0, 0, 1)
            evict_m(0, 0, 1)
            conv_m(0, 0, 0)
            evict_m(0, 0, 0)
            conv_m(0, 1, 3)
            evict_m(0, 1, 3)
            j7_stats_mm(0)
            conv_m(0, 1, 2)
            evict_m(0, 1, 2)
            conv_m(0, 1, 1)
            evict_m(0, 1, 1)
            conv_m(0, 1, 0)
            evict_m(0, 1, 0)
            front_hi(0)
            front_lo(0)
            conv_m(1, 0, 3)
            evict_m(1, 0, 3)
            conv_m(1, 0, 2)
            evict_m(1, 0, 2)
            conv_m(1, 1, 3)
            evict_m(1, 1, 3)
            j7_stats_mm(1)
            conv_m(1, 1, 2)
            evict_m(1, 1, 2)
            front_hi(1)
            j7_mid_a()
            conv_m(1, 0, 1)
            evict_m(1, 0, 1)
            conv_m(1, 0, 0)
            evict_m(1, 0, 0)
            conv_m(1, 1, 1)
            evict_m(1, 1, 1)
            conv_m(1, 1, 0)
            evict_m(1, 1, 0)
            j7_mid_b()
            j7_mid_c()
            front_lo(1)
            chain4()
            tail4()

    nc.compile()
    return nc


# cB = sum(arow) -- a host-side constant baked into the kernel IR.
CB_SUM_AROW = [0.0]

_NC_CACHE = {}


def _get_nc():
    if "nc" not in _NC_CACHE:
        _NC_CACHE["nc"] = build_nc()
    return _NC_CACHE["nc"]


def _prep_in_maps(inputs):
    f32 = np.float32
    X = np.ascontiguousarray(inputs["X"], f32)
    conv_w = np.asarray(inputs["conv_w"], f32)
    conv_b = np.asarray(inputs["conv_b"], f32)
    qw, kw, vw, ow = (np.asarray(inputs[k], f32) for k in ("qw", "kw", "vw", "ow"))
    expert_w = np.asarray(inputs["expert_w"], f32)
    expert_b = np.asarray(inputs["expert_b"], f32)
    router_w = np.asarray(inputs["router_w"], f32)
    moe_out_w = np.asarray(inputs["moe_out_w"], f32)
    out_w = np.asarray(inputs["out_w"], f32)

    nb = np.dtype(mybir.dt.np(BF16))
    n8 = np.dtype(mybir.dt.np(FP8E4))
    n5 = np.dtype(mybir.dt.np(FP8E5))
    nh = np.dtype(mybir.dt.np(FP16))

    # --- conv weights: fp8 hi (64x scale), 1x for the Xl stream, e5m2 resid
    Wc = conv_w.transpose(2, 1, 0).reshape(C * P, D)  # rows (p*64+c) -> k*128+r
    def chunked(w):
        return np.ascontiguousarray(
            w.reshape(NCH, 128, D).transpose(1, 0, 2).reshape(128, NCH * D))
    W8 = (64.0 * Wc).astype(n8)
    Wr8 = (64.0 * Wc - W8.astype(f32)).astype(n5)
    Wd8 = Wc.astype(n8)
    C8 = np.zeros((128, C8W), n8)
    C8[:, C8_W8:C8_W8 + 768] = chunked(W8.astype(f32)).astype(n8)
    C8[:, C8_WD:C8_WD + 768] = chunked(Wd8.astype(f32)).astype(n8)
    C5 = chunked(Wr8.astype(f32)).astype(n5)

    # --- CB (bf16) ---
    CB = np.zeros((128, CBW), f32)
    A_T = qw.T @ kw                    # lhsT for q0 = A @ h0_last
    CB[:, CB_AT:CB_AT + D] = np.concatenate([A_T, A_T], axis=0)
    vwT = vw.T
    CB[:, CB_VWT:CB_VWT + D] = np.concatenate([vwT, vwT], axis=0)
    CB[0:64, CB_SEL] = 1.0
    CB[64:128, CB_SEL + 1] = 1.0
    arow = (kw.T @ qw.sum(1)) / 8.0    # A @ 1, with the 1/sqrt(D) folded in
    CB[:, CB_AROW] = np.concatenate([arow, arow], axis=0)
    CB[:, CB_ONE] = 1.0
    # moe_out_w folded into the expert weights and biases
    Wme = np.einsum('ik,ekj->eij', moe_out_w, expert_w)
    # Wme[e, o2, d_in] = sum_o moe_out_w[o2, o] * expert_w[e, o, d_in]
    bme = expert_b @ moe_out_w.T       # (E, D)
    WexpE = np.concatenate(
        [Wme.transpose(2, 0, 1).reshape(D, E * D),
         bme.reshape(1, E * D)], axis=0)
    CB[0:D + 1, CB_WEXP:CB_WEXP + E * D] = WexpE
    CB_SUM_AROW[0] = float(arow.sum())

    # --- PB (fp16): 64*(pe + conv_b).T doubled rows ---
    PB = np.zeros((128, PBW), f32)
    pebT = 64.0 * (_pos_encoding_np(N, D) + conv_b[None, :]).T  # (64, N)
    PB[0:64, 0:N] = pebT
    PB[64:128, 0:N] = pebT

    # --- CF (f32) ---
    CF = np.zeros((128, CFW), f32)
    CF[0:D, CF_OWT:CF_OWT + D] = ow.T
    CF[0:D, CF_OUTWT:CF_OUTWT + PRED] = out_w.T
    CF[0:D, CF_RWT:CF_RWT + E] = router_w.T
    CF[0:D, CF_SV] = vw.sum(1)
    CF[126, CF_OH] = 1.0
    CF[0:4, CF_ID4:CF_ID4 + 4] = np.eye(4, dtype=f32)
    CF[0:8, CF_ID8:CF_ID8 + 8] = np.eye(8, dtype=f32)
    CF[0:1, CF_ONESR:CF_ONESR + 128] = 1.0
    CF[0:D, CF_ONE64] = 1.0
    ocs = out_w.T.sum(0)               # (96,)
    CF[0:SPC, CF_OCS:CF_OCS + PRED] = np.stack([ocs] * SPC, axis=0)
    CF[:, CF_LASTM] = 1.0
    CF[127, CF_LASTM] = 0.0

    # --- X: host-deinterleaved even/odd columns, fp8 hi + 64x residual ---
    Xr = X.reshape(B, C, LH, 2)
    common = dict(
        CB=np.ascontiguousarray(CB).astype(nb),
        C8=np.ascontiguousarray(C8),
        C5=np.ascontiguousarray(C5),
        PB=np.ascontiguousarray(PB).astype(nh),
        CF=np.ascontiguousarray(CF),
    )
    in_maps = []
    for c in range(NCORES):
        m = dict(common)
        xr = Xr[c * SPC:(c + 1) * SPC]
        xd = np.concatenate([xr[..., 0], xr[..., 1]], axis=1)  # (SPC, 128, LH)
        xd = np.ascontiguousarray(xd)
        xh8 = xd.astype(n8)
        m["Xh"] = xh8
        # last-token residual: Xd cols 6132:6144 per sample -> [128, 4*12]
        xl = 64.0 * (xd[:, :, LH - 12:LH] - xh8[:, :, LH - 12:LH].astype(f32))
        m["XlL"] = np.ascontiguousarray(
            xl.transpose(1, 0, 2).reshape(128, SPC * 12)).astype(n8)
        in_maps.append(m)
    return in_maps


def kernel(**inputs) -> np.ndarray:
    in_maps = _prep_in_maps(inputs)
    nc = _get_nc()
    res = run_bass_kernel_spmd(nc, in_maps, core_ids=list(range(NCORES)))
    out = np.concatenate([res.results[c]["Yout"] for c in range(NCORES)], axis=0)
    return out.astype(np.float32)
